# revision 10
# baseline (speedup 1.0000x reference)
"""NeuroStockBloom GNN kernel for 8 Trainium2 NeuronCores.

Strategy
--------
Data-parallel over the 200000 sentence nodes: 25000 sentences per core
(padded to 25088 = 49*4*128). Company side (617 nodes, padded to 640) is
replicated.

Graph aggregations are dense matmuls against count matrices built on the
host from the edge lists (small integers, exact in fp8 e4m3):
  - s2c: msg12[g, c] += scat[s, g]^T cnt1[s, c]  (both GIN layers fused
    in one [128, 640] PSUM accumulation; one AllReduce, split in two
    halves so the first overlaps the second half of the stream)
  - c2s: msg_s[s, f]  = cnt2[c, s]^T comp0[c, f]
  - c2c: resident [640, 640] fp8 count matrix.

Counts stream as fp8 (exact); all value tensors are bf16 (fp8 values
cost ~3% relative error, which random-sign sums do NOT average away).
Host-swizzled layouts make each DMA one large contiguous transfer.

Bias folding: proj_b is folded out of the sentence stream; its effect on
the layer-1 s2c message is restored with a rank-1 deg1 x proj_b term on
the company side, and on the sentence update via b02' = b02 + W02 @
proj_b, which rides row 64 of the augmented transpose (ones column).
LSTM biases are folded into the scalar-engine activations (per-partition
bias operand), removing 4 matmuls per step; LSTM weight matmuls run in
bf16 (state stays f32).

The GNN stream loop is software-pipelined (stages lagged by 1-3 chunks)
so every PE instruction's DVE/scalar-produced inputs are ready chunks in
advance - the in-order PE queue never stalls on the per-chunk
vector/scalar round trip, which also keeps the PE HAM-warm.
"""

import sys

import numpy as np
import ml_dtypes

import concourse.bass as bass
import concourse.bacc as bacc
import concourse.mybir as mybir
import concourse.tile as tile
from concourse.bass_utils import run_bass_kernel_spmd
from concourse.masks import make_identity

NCORES = 8
NC, S, T, F, D, A = 617, 200000, 15, 5, 64, 768
SS = S // NCORES          # 25000 sentences per core
SP = 25088                # padded (49 superchunks of 4 chunks of 128)
NSC = 49                  # superchunks per core
NCH = 196                 # chunks per core
CP = 640                  # padded companies (5 blocks of 128)
CW = CP // NCORES         # 80 companies per core for the LSTM branch
f32 = mybir.dt.float32
bf16 = mybir.dt.bfloat16
f8 = mybir.dt.float8e4
BF = ml_dtypes.bfloat16
F8 = ml_dtypes.float8_e4m3
AF = mybir.ActivationFunctionType
OP = mybir.AluOpType

AR_SPLIT_J = 100          # chunks 0..99 -> AllReduce #1, 100..195 -> #2


def _build(a_vals):
    """Build the SPMD bass program. a_vals: [2, 3] PReLU slopes."""
    a_vals = np.asarray(a_vals, np.float64)
    assert np.all(a_vals >= 0) and np.all(a_vals <= 1), "prelu-as-max needs 0<=a<=1"

    nc = bacc.Bacc("TRN2", target_bir_lowering=False, debug=False,
                   num_devices=NCORES)

    # ---- per-core inputs ----
    xq_d = nc.dram_tensor("xq", [128, NSC, 4, 6, 128], bf16, kind="ExternalInput")
    c1t_d = nc.dram_tensor("c1t", [128, NSC, 4, CP], f8, kind="ExternalInput")
    c2t_d = nc.dram_tensor("c2t", [128, NSC, 4, 5, 128], f8, kind="ExternalInput")
    tsmy = nc.dram_tensor("tsmy", [CW, T], f32, kind="ExternalInput")
    cembmy = nc.dram_tensor("cembmy", [D, CW], f32, kind="ExternalInput")
    # ---- replicated inputs ----
    c3t_d = nc.dram_tensor("c3t8", [128, 5, CP], f8, kind="ExternalInput")
    tsq = nc.dram_tensor("tsq", [CP, T], f32, kind="ExternalInput")
    bn_g = nc.dram_tensor("bn_g", [T, 1], f32, kind="ExternalInput")
    bn_b = nc.dram_tensor("bn_b", [T, 1], f32, kind="ExternalInput")
    wih0r = nc.dram_tensor("wih0r", [1, 4 * D], bf16, kind="ExternalInput")
    b0col = nc.dram_tensor("b0col", [D, 4], f32, kind="ExternalInput")
    whh0 = nc.dram_tensor("whh0", [D, 4 * D], bf16, kind="ExternalInput")
    wih1t = nc.dram_tensor("wih1t", [D, 4 * D], bf16, kind="ExternalInput")
    b1col = nc.dram_tensor("b1col", [D, 4], f32, kind="ExternalInput")
    whh1 = nc.dram_tensor("whh1", [D, 4 * D], bf16, kind="ExternalInput")
    fcw = nc.dram_tensor("fcw", [D, D], f32, kind="ExternalInput")
    fcb = nc.dram_tensor("fcb", [1, D], f32, kind="ExternalInput")
    wprojr = nc.dram_tensor("wprojr", [128, 6, D], bf16, kind="ExternalInput")
    gw02aug = nc.dram_tensor("gw02aug", [D + 1, D], bf16, kind="ExternalInput")
    gw = {}
    gb = {}
    for l in range(2):
        for r in range(2):
            gw[(l, r)] = nc.dram_tensor(f"gw{l}{r}", [D, D], bf16,
                                        kind="ExternalInput")
            gb[(l, r)] = nc.dram_tensor(f"gb{l}{r}", [1, D], bf16,
                                        kind="ExternalInput")
    clsw = nc.dram_tensor("clsw", [D, 2], f32, kind="ExternalInput")
    clsb = nc.dram_tensor("clsb", [1, 2], f32, kind="ExternalInput")
    deg1row = nc.dram_tensor("deg1row", [1, CP], f32, kind="ExternalInput")
    projbcol = nc.dram_tensor("projbcol", [1, D], f32, kind="ExternalInput")
    ones128 = nc.dram_tensor("ones128", [128, 1], f32, kind="ExternalInput")
    epsd = nc.dram_tensor("epsd", [T, 1], f32, kind="ExternalInput")
    onesrow = nc.dram_tensor("onesrow", [1, CP], f32, kind="ExternalInput")
    onesrowb = nc.dram_tensor("onesrowb", [1, CP], bf16, kind="ExternalInput")
    out_d = nc.dram_tensor("out", [2, CP], f32, kind="ExternalOutput")

    a02 = float(a_vals[0, 2])

    with tile.TileContext(nc) as tc:
        with (
            tc.tile_pool(name="const", bufs=1) as cpool,
            tc.tile_pool(name="res", bufs=1) as rpool,
            tc.tile_pool(name="xs", bufs=5) as xpool,
            tc.tile_pool(name="c1s", bufs=5) as c1pool,
            tc.tile_pool(name="c2s", bufs=5) as c2pool,
            tc.tile_pool(name="wk", bufs=4) as wk,
            tc.tile_pool(name="lstm", bufs=1) as lp,
            tc.tile_pool(name="dram", bufs=1, space="DRAM") as dpool,
        ):
            # ---------- constants ----------
            ident = cpool.tile([128, 128], f32)
            make_identity(nc, ident[:])
            ones_c = cpool.tile([128, 1], f32, tag="onesc")
            nc.sync.dma_start(out=ones_c[:], in_=ones128[:])
            ones_r = cpool.tile([1, CP], f32, tag="onesr")
            nc.sync.dma_start(out=ones_r[:], in_=onesrow[:])
            ones_rb = cpool.tile([1, CP], bf16, tag="onesrb")
            nc.sync.dma_start(out=ones_rb[:], in_=onesrowb[:])

            def load_const(name, dram, shape, dtype=f32):
                t = cpool.tile(shape, dtype, tag=name, name=name)
                nc.sync.dma_start(out=t[:], in_=dram[:])
                return t

            wih0r_s = load_const("wih0r", wih0r, [1, 4 * D], bf16)
            b0c_s = load_const("b0col", b0col, [D, 4])
            whh0_s = load_const("whh0", whh0, [D, 4 * D], bf16)
            wih1t_s = load_const("wih1t", wih1t, [D, 4 * D], bf16)
            b1c_s = load_const("b1col", b1col, [D, 4])
            whh1_s = load_const("whh1", whh1, [D, 4 * D], bf16)
            fcw_s = load_const("fcw", fcw, [D, D])
            fcb_s = load_const("fcb", fcb, [1, D])
            wprojr_s = load_const("wprojr", wprojr, [128, 6, D], bf16)
            gw02aug_s = load_const("gw02aug", gw02aug, [D + 1, D], bf16)
            gw_s = {k: load_const(f"gw{k[0]}{k[1]}", v, [D, D], bf16)
                    for k, v in gw.items()}
            gb_s = {k: load_const(f"gb{k[0]}{k[1]}", v, [1, D], bf16)
                    for k, v in gb.items()}
            clsw_s = load_const("clsw", clsw, [D, 2])
            clsb_s = load_const("clsb", clsb, [1, 2])
            deg1_s = load_const("deg1row", deg1row, [1, CP])
            projb_s = load_const("projbcol", projbcol, [1, D])
            eps_s = load_const("epsd", epsd, [T, 1])
            bn_g_s = load_const("bn_g", bn_g, [T, 1])
            bn_b_s = load_const("bn_b", bn_b, [T, 1])
            cemb_s = load_const("cembmy", cembmy, [D, CW])
            c3t_s = load_const("c3t8", c3t_d, [128, 5, CP], f8)

            # persistent sentence-feature store: [sent0 | sent1] rows, bf16
            scat = rpool.tile([128, NCH, 2, D], bf16, tag="scat")

            # =========== region 1: BN stats + LSTM + projection loop ====
            with tc.tile_pool(name="ps1", bufs=1, space="PSUM") as pR1:
                # ---------- BatchNorm stats (replicated, tiny) ----------
                tsch = wk.tile([128, 5, T], f32, tag="tsch", bufs=1)
                nc.sync.dma_start(
                    out=tsch[:], in_=tsq.ap().rearrange("(q p) t -> p q t", p=128))
                sq = wk.tile([128, 5, T], f32, tag="tssq", bufs=1)
                nc.vector.tensor_mul(out=sq[:], in0=tsch[:], in1=tsch[:])
                psums = pR1.tile([T, 2], f32, tag="ls", bufs=2)
                for q in range(5):
                    nc.tensor.matmul(out=psums[:, 0:1], lhsT=tsch[:, q, :],
                                     rhs=ones_c[:], start=(q == 0), stop=(q == 4))
                for q in range(5):
                    nc.tensor.matmul(out=psums[:, 1:2], lhsT=sq[:, q, :],
                                     rhs=ones_c[:], start=(q == 0), stop=(q == 4))
                mean = wk.tile([T, 1], f32, tag="mean", bufs=1)
                nc.scalar.mul(out=mean[:], in_=psums[:, 0:1], mul=1.0 / NC)
                msq = wk.tile([T, 1], f32, tag="msq", bufs=1)
                nc.vector.tensor_mul(out=msq[:], in0=mean[:], in1=mean[:])
                var = wk.tile([T, 1], f32, tag="var", bufs=1)
                nc.scalar.mul(out=var[:], in_=psums[:, 1:2], mul=1.0 / NC)
                nc.vector.tensor_sub(out=var[:], in0=var[:], in1=msq[:])
                nc.vector.tensor_add(out=var[:], in0=var[:], in1=eps_s[:])
                sd = wk.tile([T, 1], f32, tag="sd", bufs=1)
                nc.scalar.activation(out=sd[:], in_=var[:], func=AF.Sqrt)
                inv = wk.tile([T, 1], f32, tag="inv", bufs=1)
                nc.vector.reciprocal(out=inv[:], in_=sd[:])
                scale = wk.tile([T, 1], f32, tag="scale", bufs=1)
                nc.vector.tensor_mul(out=scale[:], in0=bn_g_s[:], in1=inv[:])
                mscaled = wk.tile([T, 1], f32, tag="mscaled", bufs=1)
                nc.vector.tensor_mul(out=mscaled[:], in0=mean[:], in1=scale[:])
                shift = wk.tile([T, 1], f32, tag="shift", bufs=1)
                nc.vector.tensor_sub(out=shift[:], in0=bn_b_s[:], in1=mscaled[:])

                # ---------- normalize this core's LSTM slice ----------
                tsmy_s = wk.tile([CW, T], f32, tag="tsmy", bufs=1)
                nc.sync.dma_start(out=tsmy_s[:], in_=tsmy[:])
                ptsT = pR1.tile([T, CW], f32, tag="ls", bufs=2)
                nc.tensor.transpose(out=ptsT[:], in_=tsmy_s[:],
                                    identity=ident[:CW, :CW])
                tsn = lp.tile([T, CW], f32, tag="tsn")
                nc.vector.tensor_tensor(out=tsn[:], in0=ptsT[:],
                                        in1=scale[:].to_broadcast([T, CW]),
                                        op=OP.mult)
                nc.vector.tensor_tensor(out=tsn[:], in0=tsn[:],
                                        in1=shift[:].to_broadcast([T, CW]),
                                        op=OP.add)
                # flatten to one partition (bf16) so step t is a row slice
                tsnb = lp.tile([T, CW], bf16, tag="tsnb")
                nc.vector.tensor_copy(out=tsnb[:], in_=tsn[:])
                tsf = lp.tile([1, T, CW], bf16, tag="tsf")
                nc.sync.dma_start(out=tsf[:], in_=tsnb[:])

                # ---------- LSTM state (f32) + bf16 matmul views ----------
                h0 = lp.tile([D, CW], f32, tag="h0")
                nc.vector.memset(h0[:], 0.0)
                c0 = lp.tile([D, CW], f32, tag="c0")
                nc.vector.memset(c0[:], 0.0)
                h1 = lp.tile([D, CW], f32, tag="h1")
                nc.vector.memset(h1[:], 0.0)
                c1 = lp.tile([D, CW], f32, tag="c1")
                nc.vector.memset(c1[:], 0.0)
                h0b = lp.tile([D, CW], bf16, tag="h0b")
                nc.vector.memset(h0b[:], 0.0)
                h1b = lp.tile([D, CW], bf16, tag="h1b")
                nc.vector.memset(h1b[:], 0.0)

                def lstm_step(hb, c_st, w_hh, w_ih, x_ap, b_col, h_out, hb_out):
                    # gates: pg[:, k, :] = Whh[:,k] @ hb + Wih[:,k] @ x
                    pg = pR1.tile([D, 4, CW], f32, tag="ls", bufs=2)
                    for k in range(4):
                        sl = slice(D * k, D * (k + 1))
                        nc.tensor.matmul(out=pg[:, k, :], lhsT=w_hh[:, sl],
                                         rhs=hb[:], start=True, stop=False)
                        nc.tensor.matmul(out=pg[:, k, :], lhsT=w_ih[:, sl],
                                         rhs=x_ap, start=False, stop=True)
                    # biases folded into the activations (per-partition)
                    si = lp.tile([D, CW], f32, tag="si")
                    nc.scalar.activation(out=si[:], in_=pg[:, 0, :],
                                         func=AF.Sigmoid, bias=b_col[:, 0:1])
                    sf = lp.tile([D, CW], f32, tag="sf")
                    nc.scalar.activation(out=sf[:], in_=pg[:, 1, :],
                                         func=AF.Sigmoid, bias=b_col[:, 1:2])
                    tg = lp.tile([D, CW], f32, tag="tg")
                    nc.scalar.activation(out=tg[:], in_=pg[:, 2, :],
                                         func=AF.Tanh, bias=b_col[:, 2:3])
                    so = lp.tile([D, CW], f32, tag="so")
                    nc.scalar.activation(out=so[:], in_=pg[:, 3, :],
                                         func=AF.Sigmoid, bias=b_col[:, 3:4])
                    t1 = lp.tile([D, CW], f32, tag="t1")
                    nc.vector.tensor_mul(out=t1[:], in0=si[:], in1=tg[:])
                    nc.vector.tensor_mul(out=c_st[:], in0=sf[:], in1=c_st[:])
                    nc.vector.tensor_add(out=c_st[:], in0=c_st[:], in1=t1[:])
                    tcs = lp.tile([D, CW], f32, tag="tcs")
                    nc.scalar.activation(out=tcs[:], in_=c_st[:], func=AF.Tanh)
                    nc.vector.tensor_mul(out=h_out[:], in0=so[:], in1=tcs[:])
                    nc.vector.tensor_copy(out=hb_out[:], in_=h_out[:])

                lstm_units = []
                for t in range(T):
                    lstm_units.append(
                        (lambda t=t: lstm_step(h0b, c0, whh0_s, wih0r_s,
                                               tsf[:, t, :], b0c_s, h0, h0b)))
                    lstm_units.append(
                        (lambda t=t: lstm_step(h1b, c1, whh1_s, wih1t_s,
                                               h0b[:], b1c_s, h1, h1b)))

                ag_in = dpool.tile([CW, D], f32, tag="ag_in")
                ag_out = dpool.tile([CP, D], f32, tag="ag_out")
                comp0r = rpool.tile([128, 5, D], f32, tag="comp0r")
                comp0rb = rpool.tile([128, 5, D], bf16, tag="comp0rb")

                def lstm_finish():
                    # comp_my = relu(fc(h1)) + emb -> rows [CW, 64] -> AllGather
                    pfc = pR1.tile([D, CW], f32, tag="ls", bufs=2)
                    nc.tensor.matmul(out=pfc[:], lhsT=fcw_s[:], rhs=h1[:],
                                     start=True, stop=False)
                    nc.tensor.matmul(out=pfc[:], lhsT=fcb_s[:],
                                     rhs=ones_r[:, :CW], start=False, stop=True)
                    cmT = lp.tile([D, CW], f32, tag="cmT")
                    nc.scalar.activation(out=cmT[:], in_=pfc[:], func=AF.Relu)
                    nc.vector.tensor_add(out=cmT[:], in0=cmT[:], in1=cemb_s[:])
                    pcr = pR1.tile([CW, D], f32, tag="ls", bufs=2)
                    nc.tensor.transpose(out=pcr[:], in_=cmT[:],
                                        identity=ident[:D, :D])
                    cmr = lp.tile([CW, D], f32, tag="cmr")
                    nc.vector.tensor_copy(out=cmr[:], in_=pcr[:])
                    nc.sync.dma_start(out=ag_in[:], in_=cmr[:])
                    nc.gpsimd.collective_compute(
                        "AllGather", OP.bypass,
                        replica_groups=[list(range(NCORES))],
                        ins=[ag_in.opt()], outs=[ag_out.opt()])
                    # comp0 rows for the stream loop, issued right away so
                    # loop B can start the moment the AllGather lands
                    nc.sync.dma_start(
                        out=comp0r[:],
                        in_=ag_out[:].rearrange("(q p) f -> p q f", p=128))
                    nc.vector.tensor_copy(out=comp0rb[:], in_=comp0r[:])

                lstm_units.append(lstm_finish)

                # ---------- projection loop (+LSTM interleave, 3/sc) ----
                # LSTM units go FIRST so their scalar/vector ops are not
                # queued behind this superchunk's scat casts
                ui = 0
                for sc in range(NSC):
                    for _ in range(3):
                        if ui < len(lstm_units):
                            lstm_units[ui]()
                            ui += 1
                    xt_t = xpool.tile([128, 4, 6, 128], bf16, tag="xq")
                    nc.sync.dma_start(out=xt_t[:], in_=xq_d[:, sc])
                    for c in range(4):
                        j = sc * 4 + c
                        psA = pR1.tile([128, D], f32, tag="a", bufs=3)
                        for q in range(6):
                            nc.tensor.matmul(out=psA[:], lhsT=xt_t[:, c, q, :],
                                             rhs=wprojr_s[:, q, :],
                                             start=(q == 0), stop=(q == 5))
                        nc.vector.tensor_copy(out=scat[:, j, 0, :], in_=psA[:])
                while ui < len(lstm_units):
                    lstm_units[ui]()
                    ui += 1

            # ---------- company-update helpers (pool-parametric) --------
            def t_to_rows(srcT, tag, pool):
                # [64, 640] -> bf16 rows [128, 5, 64]
                rowsb = rpool.tile([128, 5, D], bf16, tag=tag + "b",
                                   name=tag + "b")
                for q in range(5):
                    pt = pool.tile([128, D], f32, tag="small", bufs=2,
                                   name="pt")
                    nc.tensor.transpose(out=pt[:],
                                        in_=srcT[:, 128 * q:128 * (q + 1)],
                                        identity=ident[:D, :D])
                    nc.vector.tensor_copy(out=rowsb[:, q, :], in_=pt[:])
                return rowsb

            def rows_to_t(rows_f32, tag, pool):
                # f32 rows [128, 5, 64] -> [64, 640]
                dstT = rpool.tile([D, CP], f32, tag=tag, name=tag)
                for q in range(5):
                    pt = pool.tile([D, 128], f32, tag="small", bufs=2,
                                   name="pt")
                    nc.tensor.transpose(out=pt[:], in_=rows_f32[:, q, :],
                                        identity=ident[:])
                    nc.vector.tensor_copy(out=dstT[:, 128 * q:128 * (q + 1)],
                                          in_=pt[:])
                return dstT

            def c2c_msg(rowsb, pool):
                p = pool.tile([D, CP], f32, tag="big", bufs=2, name="pc2c")
                for q in range(5):
                    for sl, sz in ((0, 512), (512, 128)):
                        nc.tensor.matmul(out=p[:, sl:sl + sz],
                                         lhsT=rowsb[:, q, :],
                                         rhs=c3t_s[:, q, sl:sl + sz],
                                         start=(q == 0), stop=(q == 4))
                return p

            def gin_update(compT, msg_ap, l, r, tag, pool):
                y = wk.tile([D, CP], bf16, tag=f"y{tag}", name=f"y{tag}",
                            bufs=1)
                nc.vector.tensor_tensor(out=y[:], in0=msg_ap, in1=compT[:],
                                        op=OP.add)
                ph = pool.tile([D, CP], f32, tag="big", bufs=2, name="ph")
                for sl, sz in ((0, 512), (512, 128)):
                    nc.tensor.matmul(out=ph[:, sl:sl + sz],
                                     lhsT=gw_s[(l, r)][:],
                                     rhs=y[:, sl:sl + sz],
                                     start=True, stop=False)
                    nc.tensor.matmul(out=ph[:, sl:sl + sz],
                                     lhsT=gb_s[(l, r)][:],
                                     rhs=ones_rb[:, sl:sl + sz],
                                     start=False, stop=True)
                o = wk.tile([D, CP], f32, tag=f"o{tag}", name=f"o{tag}",
                            bufs=1)
                av = float(a_vals[l, r])
                nc.scalar.mul(out=o[:], in_=ph[:], mul=av)
                nc.vector.tensor_max(out=o[:], in0=o[:], in1=ph[:])
                return o

            # ===== region 1.5: AR-independent company work ==============
            # (gated only on the AllGather; overlaps the loop A -> B seam)
            with tc.tile_pool(name="ps15", bufs=1, space="PSUM") as pR15:
                pcorr_ps = pR15.tile([D, CP], f32, tag="big", bufs=2)
                for sl, sz in ((0, 512), (512, 128)):
                    nc.tensor.matmul(out=pcorr_ps[:, sl:sl + sz],
                                     lhsT=projb_s[:],
                                     rhs=deg1_s[:, sl:sl + sz],
                                     start=True, stop=True)
                pcorr = rpool.tile([D, CP], f32, tag="pcorr")
                nc.vector.tensor_copy(out=pcorr[:], in_=pcorr_ps[:])
                comp0T = rows_to_t(comp0r, "comp0T", pR15)
                pc2c0 = c2c_msg(comp0rb, pR15)
                ha = gin_update(comp0T, pc2c0[:], 0, 0, "a1", pR15)

            ar_in_a = dpool.tile([128, CP], f32, tag="ar_in_a")
            ar_out_a = dpool.tile([128, CP], f32, tag="ar_out_a")
            ar_in_b = dpool.tile([128, CP], f32, tag="ar_in_b")
            ar_out_b = dpool.tile([128, CP], f32, tag="ar_out_b")

            # =========== region 2: GNN streaming loop (sw-pipelined) ====
            with tc.tile_pool(name="ps2", bufs=1, space="PSUM") as pR2:
                msg12a = pR2.tile([128, CP], f32, tag="ma", bufs=1)
                msg12b = pR2.tile([128, CP], f32, tag="mb", bufs=1)

                def flush_half(mm, ar_in, ar_out, tag):
                    m_sb = rpool.tile([128, CP], f32, tag=tag, name=tag)
                    nc.vector.tensor_copy(out=m_sb[:], in_=mm[:])
                    nc.sync.dma_start(out=ar_in[:], in_=m_sb[:])
                    nc.gpsimd.collective_compute(
                        "AllReduce", OP.add,
                        replica_groups=[list(range(NCORES))],
                        ins=[ar_in.opt()], outs=[ar_out.opt()])

                c1_tiles = {}
                xs_tiles = {}
                xtb_tiles = {}
                ps1_tiles = {}
                pst_tiles = {}

                def stage_a(j):
                    # c2s message + xs (sent0 + msg, ones column)
                    sc, c = j // 4, j % 4
                    c2_t = c2_tiles[sc]
                    psB = pR2.tile([128, D], f32, tag="bp", bufs=3)
                    for q in range(5):
                        nc.tensor.matmul(out=psB[:], lhsT=c2_t[:, c, q, :],
                                         rhs=comp0rb[:, q, :],
                                         start=(q == 0), stop=(q == 4))
                    xs_aug = wk.tile([128, D + 1], f32, tag="xsa")
                    nc.vector.tensor_tensor(out=xs_aug[:, 0:D], in0=psB[:],
                                            in1=scat[:, j, 0, :], op=OP.add)
                    nc.vector.memset(xs_aug[:, D:D + 1], 1.0)
                    xs_tiles[j] = xs_aug

                def stage_b(j):
                    psT = pR2.tile([D + 1, 128], f32, tag="t", bufs=1)
                    nc.tensor.transpose(out=psT[:], in_=xs_tiles[j][:],
                                        identity=ident[:])
                    xTb = wk.tile([D + 1, 128], bf16, tag="xtb")
                    nc.vector.tensor_copy(out=xTb[:], in_=psT[:])
                    xtb_tiles[j] = xTb
                    del xs_tiles[j]

                def stage_c(j):
                    # sent1 = prelu(xs @ W02.T + b02')
                    ps1 = pR2.tile([128, D], f32, tag="bp", bufs=3)
                    nc.tensor.matmul(out=ps1[:], lhsT=xtb_tiles[j][:],
                                     rhs=gw02aug_s[:], start=True, stop=True)
                    pr = wk.tile([128, D], f32, tag="pr")
                    nc.vector.tensor_scalar_mul(pr[:], ps1[:], a02)
                    nc.vector.tensor_max(out=scat[:, j, 1, :],
                                         in0=pr[:], in1=ps1[:])
                    del xtb_tiles[j]

                def stage_d(j):
                    # fused s2c for both layers
                    sc, c = j // 4, j % 4
                    c1_t = c1_tiles[sc]
                    mm = msg12a if j < AR_SPLIT_J else msg12b
                    j_first = 0 if j < AR_SPLIT_J else AR_SPLIT_J
                    j_last = AR_SPLIT_J - 1 if j < AR_SPLIT_J else NCH - 1
                    for sl, sz in ((0, 512), (512, 128)):
                        nc.tensor.matmul(out=mm[:, sl:sl + sz],
                                         lhsT=scat[:, j, :, :],
                                         rhs=c1_t[:, c, sl:sl + sz],
                                         start=(j == j_first),
                                         stop=(j == j_last))
                    if j == AR_SPLIT_J - 1:
                        flush_half(msg12a, ar_in_a, ar_out_a, "m12a")

                c2_tiles = {}
                for sc in range(NSC):
                    c1_t = c1pool.tile([128, 4, CP], f8, tag="c1")
                    nc.sync.dma_start(out=c1_t[:], in_=c1t_d[:, sc])
                    c1_tiles[sc] = c1_t
                    c2_t = c2pool.tile([128, 4, 5, 128], f8, tag="c2")
                    nc.sync.dma_start(out=c2_t[:], in_=c2t_d[:, sc])
                    c2_tiles[sc] = c2_t
                    for c in range(4):
                        j = sc * 4 + c
                        stage_a(j)
                        if j >= 1:
                            stage_b(j - 1)
                        if j >= 2:
                            stage_c(j - 2)
                        if j >= 3:
                            stage_d(j - 3)
                stage_b(NCH - 1)
                stage_c(NCH - 2)
                stage_c(NCH - 1)
                for j in (NCH - 3, NCH - 2, NCH - 1):
                    stage_d(j)
                flush_half(msg12b, ar_in_b, ar_out_b, "m12b")

            m1a = rpool.tile([D, CP], f32, tag="m1a")
            nc.sync.dma_start(out=m1a[:], in_=ar_out_a[:D, :])
            m1b = rpool.tile([D, CP], f32, tag="m1b")
            nc.sync.dma_start(out=m1b[:], in_=ar_out_b[:D, :])
            m2a = rpool.tile([D, CP], f32, tag="m2a")
            nc.sync.dma_start(out=m2a[:], in_=ar_out_a[D:, :])
            m2b = rpool.tile([D, CP], f32, tag="m2b")
            nc.sync.dma_start(out=m2b[:], in_=ar_out_b[D:, :])

            # =========== region 3: AR-dependent company updates =========
            with tc.tile_pool(name="ps3", bufs=1, space="PSUM") as pR3:
                msgr1 = rpool.tile([D, CP], f32, tag="msgr1")
                nc.vector.tensor_add(out=msgr1[:], in0=m1a[:], in1=m1b[:])
                nc.vector.tensor_add(out=msgr1[:], in0=msgr1[:], in1=pcorr[:])
                msgr2 = rpool.tile([D, CP], f32, tag="msgr2")
                nc.vector.tensor_add(out=msgr2[:], in0=m2a[:], in1=m2b[:])

                # layer 1 (ha precomputed in region 1.5)
                hb = gin_update(comp0T, msgr1[:], 0, 1, "b1", pR3)
                comp1T = rpool.tile([D, CP], f32, tag="comp1T")
                nc.vector.tensor_add(out=comp1T[:], in0=ha[:], in1=hb[:])
                comp1rb = t_to_rows(comp1T, "comp1r", pR3)
                # layer 2
                pc2c1 = c2c_msg(comp1rb, pR3)
                ha2 = gin_update(comp1T, pc2c1[:], 1, 0, "a2", pR3)
                hb2 = gin_update(comp1T, msgr2[:], 1, 1, "b2", pR3)
                comp2T = rpool.tile([D, CP], f32, tag="comp2T")
                nc.vector.tensor_add(out=comp2T[:], in0=ha2[:], in1=hb2[:])

                # classifier (f32)
                pcls = pR3.tile([2, CP], f32, tag="big", bufs=2)
                for sl, sz in ((0, 512), (512, 128)):
                    nc.tensor.matmul(out=pcls[:, sl:sl + sz], lhsT=clsw_s[:],
                                     rhs=comp2T[:, sl:sl + sz],
                                     start=True, stop=False)
                    nc.tensor.matmul(out=pcls[:, sl:sl + sz], lhsT=clsb_s[:],
                                     rhs=ones_r[:, sl:sl + sz],
                                     start=False, stop=True)
                outs = wk.tile([2, CP], f32, tag="outs", bufs=1)
                nc.vector.tensor_copy(out=outs[:], in_=pcls[:])
                nc.sync.dma_start(out=out_d[:], in_=outs[:])

    nc.compile()
    return nc


_CACHE = {}


def _get_program(a_vals):
    key = np.asarray(a_vals, np.float64).tobytes()
    if key not in _CACHE:
        _CACHE[key] = _build(a_vals)
    return _CACHE[key]


def _prep_inputs(inp):
    """Host-side sharding, layout swizzles, count-matrix construction."""
    sx = np.asarray(inp["sentence_x"], np.float32)
    cts = np.asarray(inp["company_ts"], np.float32)
    cids = np.asarray(inp["company_ids"]).astype(np.int64)
    emb = np.asarray(inp["comp_emb"], np.float32)

    tsq = np.zeros((CP, T), np.float32)
    tsq[:NC] = cts[:, :, F - 2]
    cembT = np.zeros((D, CP), np.float32)
    cembT[:, :NC] = emb[cids].T

    s2c_s = np.asarray(inp["ei_s2c_src"]).astype(np.int64)
    s2c_d = np.asarray(inp["ei_s2c_dst"]).astype(np.int64)
    c2s_s = np.asarray(inp["ei_c2s_src"]).astype(np.int64)
    c2s_d = np.asarray(inp["ei_c2s_dst"]).astype(np.int64)
    c2c_s = np.asarray(inp["ei_c2c_src"]).astype(np.int64)
    c2c_d = np.asarray(inp["ei_c2c_dst"]).astype(np.int64)

    c3t = np.bincount(c2c_s * CP + c2c_d, minlength=CP * CP).reshape(
        CP, CP).astype(np.float32)
    # [640, 640] -> [128, 5, 640] (partition = src % 128, block = src // 128)
    c3t8 = np.ascontiguousarray(
        c3t.reshape(5, 128, CP).transpose(1, 0, 2)).astype(F8)

    deg1 = np.bincount(s2c_d, minlength=CP).astype(np.float32).reshape(1, CP)

    core1 = s2c_s // SS
    loc1 = s2c_s - core1 * SS
    core2 = c2s_d // SS
    loc2 = c2s_d - core2 * SS

    per_core = []
    for k in range(NCORES):
        m1 = core1 == k
        cnt1 = np.bincount(loc1[m1] * CP + s2c_d[m1],
                           minlength=SP * CP).reshape(SP, CP)
        c1t = np.ascontiguousarray(
            cnt1.reshape(NSC, 4, 128, CP).transpose(2, 0, 1, 3)).astype(F8)
        del cnt1
        m2 = core2 == k
        cnt2 = np.bincount(c2s_s[m2] * SP + loc2[m2],
                           minlength=CP * SP).reshape(CP, SP)
        c2t = np.ascontiguousarray(
            cnt2.reshape(5, 128, NSC, 4, 128).transpose(1, 2, 3, 0, 4)
        ).astype(F8)
        del cnt2
        xk = np.zeros((SP, A), np.float32)
        xk[:SS] = sx[SS * k:SS * (k + 1)]
        xq = np.ascontiguousarray(
            xk.reshape(NSC, 4, 128, 6, 128).transpose(4, 0, 1, 3, 2)
        ).astype(BF)
        del xk
        per_core.append({
            "xq": xq, "c1t": c1t, "c2t": c2t,
            "tsmy": np.ascontiguousarray(tsq[CW * k:CW * (k + 1)]),
            "cembmy": np.ascontiguousarray(cembT[:, CW * k:CW * (k + 1)]),
        })

    gin_W = np.asarray(inp["gin_W"], np.float32)
    gin_b = np.asarray(inp["gin_b"], np.float32)
    proj_W = np.asarray(inp["proj_W"], np.float32)
    proj_b = np.asarray(inp["proj_b"], np.float32)
    # b02' = b02 + W02 @ proj_b (proj_b folded out of the sentence stream)
    b02p = gin_b[0, 2] + gin_W[0, 2] @ proj_b
    gw02aug = np.concatenate([gin_W[0, 2].T, b02p.reshape(1, D)],
                             axis=0).astype(BF)
    wprojr = np.ascontiguousarray(
        proj_W.T.reshape(6, 128, D).transpose(1, 0, 2)).astype(BF)

    b0 = (np.asarray(inp["lstm_bih0"], np.float32)
          + np.asarray(inp["lstm_bhh0"], np.float32))
    b1 = (np.asarray(inp["lstm_bih1"], np.float32)
          + np.asarray(inp["lstm_bhh1"], np.float32))

    lw = {
        "c3t8": c3t8, "tsq": tsq,
        "bn_g": np.asarray(inp["bn_gamma"], np.float32).reshape(T, 1),
        "bn_b": np.asarray(inp["bn_beta"], np.float32).reshape(T, 1),
        "wih0r": np.asarray(inp["lstm_Wih0"], np.float32)[:, 0].reshape(
            1, 4 * D).astype(BF),
        "b0col": np.ascontiguousarray(b0.reshape(4, D).T),
        "whh0": np.ascontiguousarray(
            np.asarray(inp["lstm_Whh0"], np.float32).T).astype(BF),
        "wih1t": np.ascontiguousarray(
            np.asarray(inp["lstm_Wih1"], np.float32).T).astype(BF),
        "b1col": np.ascontiguousarray(b1.reshape(4, D).T),
        "whh1": np.ascontiguousarray(
            np.asarray(inp["lstm_Whh1"], np.float32).T).astype(BF),
        "fcw": np.ascontiguousarray(np.asarray(inp["fc_W"], np.float32).T),
        "fcb": np.asarray(inp["fc_b"], np.float32).reshape(1, D),
        "wprojr": wprojr,
        "gw02aug": gw02aug,
        "clsw": np.ascontiguousarray(np.asarray(inp["cls_W"], np.float32).T),
        "clsb": np.asarray(inp["cls_b"], np.float32).reshape(1, 2),
        "deg1row": deg1,
        "projbcol": proj_b.reshape(1, D),
        "ones128": np.ones((128, 1), np.float32),
        "epsd": np.full((T, 1), 1e-5, np.float32),
        "onesrow": np.ones((1, CP), np.float32),
        "onesrowb": np.ones((1, CP), BF),
    }
    for l in range(2):
        for r in range(2):
            lw[f"gw{l}{r}"] = np.ascontiguousarray(gin_W[l, r].T).astype(BF)
            lw[f"gb{l}{r}"] = gin_b[l, r].reshape(1, D).astype(BF)

    in_maps = [{**per_core[k], **lw} for k in range(NCORES)]
    return in_maps


def kernel(**inputs):
    inp = {k: np.asarray(v) for k, v in inputs.items()}
    a_vals = np.asarray(inp["gin_a"], np.float32)
    nc = _get_program(a_vals)
    in_maps = _prep_inputs(inp)
    res = run_bass_kernel_spmd(nc, in_maps, list(range(NCORES)))
    out = np.asarray(res.results[0]["out"])  # [2, CP]
    return np.ascontiguousarray(out.T[:NC]).astype(np.float32)


if __name__ == "__main__":
    # quick self-test against the reference
    sys.path.insert(0, "/root/problem")
    import reference

    inputs = {k: np.asarray(v) for k, v in reference.setup_inputs().items()}
    expected = np.asarray(reference.reference(**reference.setup_inputs()))
    got = kernel(**inputs)
    err = np.abs(got - expected).max() / (np.abs(expected).max() + 1e-30)
    print("Relative error:", err)


# revision 13
# speedup vs baseline: 1.1974x; 1.1974x over previous
"""NeuroStockBloom GNN kernel for 8 Trainium2 NeuronCores.

Strategy
--------
Data-parallel over the 200000 sentence nodes: 25000 sentences per core
(padded to 25088 = 49*4*128). Company side (617 nodes, padded to 640) is
replicated.

Graph aggregations are dense matmuls against count matrices built on the
host from the edge lists (small integers, exact in fp8 e4m3):
  - s2c: msg12[g, c] += scat[s, g]^T cnt1[s, c]  (both GIN layers fused
    in one [128, 640] PSUM accumulation; one AllReduce, split in two
    halves so the first overlaps the second half of the stream)
  - c2s: msg_s[s, f]  = cnt2[c, s]^T comp0[c, f]
  - c2c: resident [640, 640] fp8 count matrix.

Counts stream as fp8 (exact); all value tensors are bf16 (fp8 values
cost ~3% relative error, which random-sign sums do NOT average away).
Host-swizzled layouts make each DMA one large contiguous transfer.

Bias folding: proj_b is folded out of the sentence stream; its effect on
the layer-1 s2c message is restored with a rank-1 deg1 x proj_b term on
the company side, and on the sentence update via b02' = b02 + W02 @
proj_b, which rides row 64 of the augmented transpose (ones column).
LSTM biases are folded into the scalar-engine activations (per-partition
bias operand), removing 4 matmuls per step; LSTM weight matmuls run in
bf16 (state stays f32).

The GNN stream loop is software-pipelined (stages lagged by 1-3 chunks)
so every PE instruction's DVE/scalar-produced inputs are ready chunks in
advance - the in-order PE queue never stalls on the per-chunk
vector/scalar round trip, which also keeps the PE HAM-warm.
"""

import sys

import numpy as np
import ml_dtypes

import concourse.bass as bass
import concourse.bacc as bacc
import concourse.mybir as mybir
import concourse.tile as tile
from concourse.bass_utils import run_bass_kernel_spmd
from concourse.masks import make_identity

NCORES = 8
NC, S, T, F, D, A = 617, 200000, 15, 5, 64, 768
SS = S // NCORES          # 25000 sentences per core
SP = 25088                # padded (49 superchunks of 4 chunks of 128)
NSC = 49                  # superchunks per core
NCH = 196                 # chunks per core
CP = 640                  # padded companies (5 blocks of 128)
CW = CP // NCORES         # 80 companies per core for the LSTM branch
f32 = mybir.dt.float32
bf16 = mybir.dt.bfloat16
f8 = mybir.dt.float8e4
BF = ml_dtypes.bfloat16
F8 = ml_dtypes.float8_e4m3
AF = mybir.ActivationFunctionType
OP = mybir.AluOpType

AR_SPLIT_J = 100          # chunks 0..99 -> AllReduce #1, 100..195 -> #2


def _build(a_vals):
    """Build the SPMD bass program. a_vals: [2, 3] PReLU slopes."""
    a_vals = np.asarray(a_vals, np.float64)
    assert np.all(a_vals >= 0) and np.all(a_vals <= 1), "prelu-as-max needs 0<=a<=1"

    nc = bacc.Bacc("TRN2", target_bir_lowering=False, debug=False,
                   num_devices=NCORES)

    # ---- per-core inputs ----
    xq_d = nc.dram_tensor("xq", [128, NSC, 4, 6, 128], bf16, kind="ExternalInput")
    c1t_d = nc.dram_tensor("c1t", [128, NSC, 4, CP], f8, kind="ExternalInput")
    c2t_d = nc.dram_tensor("c2t", [128, NSC, 4, 5, 128], f8, kind="ExternalInput")
    tsmy = nc.dram_tensor("tsmy", [CW, T], f32, kind="ExternalInput")
    cembmy = nc.dram_tensor("cembmy", [D, CW], f32, kind="ExternalInput")
    # ---- replicated inputs ----
    c3t_d = nc.dram_tensor("c3t8", [128, 5, CP], f8, kind="ExternalInput")
    tsq = nc.dram_tensor("tsq", [CP, T], f32, kind="ExternalInput")
    bn_g = nc.dram_tensor("bn_g", [T, 1], f32, kind="ExternalInput")
    bn_b = nc.dram_tensor("bn_b", [T, 1], f32, kind="ExternalInput")
    wih0r = nc.dram_tensor("wih0r", [1, 4 * D], bf16, kind="ExternalInput")
    b0col = nc.dram_tensor("b0col", [D, 4], f32, kind="ExternalInput")
    whh0 = nc.dram_tensor("whh0", [D, 4 * D], bf16, kind="ExternalInput")
    wih1t = nc.dram_tensor("wih1t", [D, 4 * D], bf16, kind="ExternalInput")
    b1col = nc.dram_tensor("b1col", [D, 4], f32, kind="ExternalInput")
    whh1 = nc.dram_tensor("whh1", [D, 4 * D], bf16, kind="ExternalInput")
    fcw = nc.dram_tensor("fcw", [D, D], f32, kind="ExternalInput")
    fcb = nc.dram_tensor("fcb", [1, D], f32, kind="ExternalInput")
    wprojr = nc.dram_tensor("wprojr", [128, 6, D], bf16, kind="ExternalInput")
    gw02aug = nc.dram_tensor("gw02aug", [D + 1, D], bf16, kind="ExternalInput")
    gw = {}
    gb = {}
    for l in range(2):
        for r in range(2):
            gw[(l, r)] = nc.dram_tensor(f"gw{l}{r}", [D, D], bf16,
                                        kind="ExternalInput")
            gb[(l, r)] = nc.dram_tensor(f"gb{l}{r}", [1, D], bf16,
                                        kind="ExternalInput")
    clsw = nc.dram_tensor("clsw", [D, 2], f32, kind="ExternalInput")
    clsb = nc.dram_tensor("clsb", [1, 2], f32, kind="ExternalInput")
    deg1row = nc.dram_tensor("deg1row", [1, CP], f32, kind="ExternalInput")
    projbcol = nc.dram_tensor("projbcol", [1, D], f32, kind="ExternalInput")
    ones128 = nc.dram_tensor("ones128", [128, 1], f32, kind="ExternalInput")
    epsd = nc.dram_tensor("epsd", [T, 1], f32, kind="ExternalInput")
    onesrow = nc.dram_tensor("onesrow", [1, CP], f32, kind="ExternalInput")
    onesrowb = nc.dram_tensor("onesrowb", [1, CP], bf16, kind="ExternalInput")
    out_d = nc.dram_tensor("out", [2, CP], f32, kind="ExternalOutput")

    a02 = float(a_vals[0, 2])

    with tile.TileContext(nc) as tc:
        with (
            tc.tile_pool(name="const", bufs=1) as cpool,
            tc.tile_pool(name="res", bufs=1) as rpool,
            tc.tile_pool(name="xs", bufs=5) as xpool,
            tc.tile_pool(name="c1s", bufs=5) as c1pool,
            tc.tile_pool(name="c2s", bufs=5) as c2pool,
            tc.tile_pool(name="wk", bufs=4) as wk,
            tc.tile_pool(name="lstm", bufs=1) as lp,
            tc.tile_pool(name="dram", bufs=1, space="DRAM") as dpool,
        ):
            # ---------- constants ----------
            ident = cpool.tile([128, 128], f32)
            make_identity(nc, ident[:])
            ones_c = cpool.tile([128, 1], f32, tag="onesc")
            nc.sync.dma_start(out=ones_c[:], in_=ones128[:])
            ones_r = cpool.tile([1, CP], f32, tag="onesr")
            nc.sync.dma_start(out=ones_r[:], in_=onesrow[:])
            ones_rb = cpool.tile([1, CP], bf16, tag="onesrb")
            nc.sync.dma_start(out=ones_rb[:], in_=onesrowb[:])

            def load_const(name, dram, shape, dtype=f32):
                t = cpool.tile(shape, dtype, tag=name, name=name)
                nc.sync.dma_start(out=t[:], in_=dram[:])
                return t

            wih0r_s = load_const("wih0r", wih0r, [1, 4 * D], bf16)
            b0c_s = load_const("b0col", b0col, [D, 4])
            whh0_s = load_const("whh0", whh0, [D, 4 * D], bf16)
            wih1t_s = load_const("wih1t", wih1t, [D, 4 * D], bf16)
            b1c_s = load_const("b1col", b1col, [D, 4])
            whh1_s = load_const("whh1", whh1, [D, 4 * D], bf16)
            fcw_s = load_const("fcw", fcw, [D, D])
            fcb_s = load_const("fcb", fcb, [1, D])
            wprojr_s = load_const("wprojr", wprojr, [128, 6, D], bf16)
            gw02aug_s = load_const("gw02aug", gw02aug, [D + 1, D], bf16)
            gw_s = {k: load_const(f"gw{k[0]}{k[1]}", v, [D, D], bf16)
                    for k, v in gw.items()}
            gb_s = {k: load_const(f"gb{k[0]}{k[1]}", v, [1, D], bf16)
                    for k, v in gb.items()}
            clsw_s = load_const("clsw", clsw, [D, 2])
            clsb_s = load_const("clsb", clsb, [1, 2])
            deg1_s = load_const("deg1row", deg1row, [1, CP])
            projb_s = load_const("projbcol", projbcol, [1, D])
            eps_s = load_const("epsd", epsd, [T, 1])
            bn_g_s = load_const("bn_g", bn_g, [T, 1])
            bn_b_s = load_const("bn_b", bn_b, [T, 1])
            cemb_s = load_const("cembmy", cembmy, [D, CW])
            c3t_s = load_const("c3t8", c3t_d, [128, 5, CP], f8)

            # persistent sentence-feature store: [sent0 | sent1] rows, bf16
            scat = rpool.tile([128, NCH, 2, D], bf16, tag="scat")

            # =========== region 1: BN stats + LSTM + projection loop ====
            with tc.tile_pool(name="ps1", bufs=1, space="PSUM") as pR1:
                # ---------- BatchNorm stats (replicated, tiny) ----------
                tsch = wk.tile([128, 5, T], f32, tag="tsch", bufs=1)
                nc.sync.dma_start(
                    out=tsch[:], in_=tsq.ap().rearrange("(q p) t -> p q t", p=128))
                sq = wk.tile([128, 5, T], f32, tag="tssq", bufs=1)
                nc.vector.tensor_mul(out=sq[:], in0=tsch[:], in1=tsch[:])
                psums = pR1.tile([T, 2], f32, tag="ls", bufs=2)
                for q in range(5):
                    nc.tensor.matmul(out=psums[:, 0:1], lhsT=tsch[:, q, :],
                                     rhs=ones_c[:], start=(q == 0), stop=(q == 4))
                for q in range(5):
                    nc.tensor.matmul(out=psums[:, 1:2], lhsT=sq[:, q, :],
                                     rhs=ones_c[:], start=(q == 0), stop=(q == 4))
                mean = wk.tile([T, 1], f32, tag="mean", bufs=1)
                nc.scalar.mul(out=mean[:], in_=psums[:, 0:1], mul=1.0 / NC)
                msq = wk.tile([T, 1], f32, tag="msq", bufs=1)
                nc.vector.tensor_mul(out=msq[:], in0=mean[:], in1=mean[:])
                var = wk.tile([T, 1], f32, tag="var", bufs=1)
                nc.scalar.mul(out=var[:], in_=psums[:, 1:2], mul=1.0 / NC)
                nc.vector.tensor_sub(out=var[:], in0=var[:], in1=msq[:])
                nc.vector.tensor_add(out=var[:], in0=var[:], in1=eps_s[:])
                sd = wk.tile([T, 1], f32, tag="sd", bufs=1)
                nc.scalar.activation(out=sd[:], in_=var[:], func=AF.Sqrt)
                inv = wk.tile([T, 1], f32, tag="inv", bufs=1)
                nc.vector.reciprocal(out=inv[:], in_=sd[:])
                scale = wk.tile([T, 1], f32, tag="scale", bufs=1)
                nc.vector.tensor_mul(out=scale[:], in0=bn_g_s[:], in1=inv[:])
                mscaled = wk.tile([T, 1], f32, tag="mscaled", bufs=1)
                nc.vector.tensor_mul(out=mscaled[:], in0=mean[:], in1=scale[:])
                shift = wk.tile([T, 1], f32, tag="shift", bufs=1)
                nc.vector.tensor_sub(out=shift[:], in0=bn_b_s[:], in1=mscaled[:])

                # ---------- normalize this core's LSTM slice ----------
                tsmy_s = wk.tile([CW, T], f32, tag="tsmy", bufs=1)
                nc.sync.dma_start(out=tsmy_s[:], in_=tsmy[:])
                ptsT = pR1.tile([T, CW], f32, tag="ls", bufs=2)
                nc.tensor.transpose(out=ptsT[:], in_=tsmy_s[:],
                                    identity=ident[:CW, :CW])
                tsn = lp.tile([T, CW], f32, tag="tsn")
                nc.vector.tensor_tensor(out=tsn[:], in0=ptsT[:],
                                        in1=scale[:].to_broadcast([T, CW]),
                                        op=OP.mult)
                nc.vector.tensor_tensor(out=tsn[:], in0=tsn[:],
                                        in1=shift[:].to_broadcast([T, CW]),
                                        op=OP.add)
                # flatten to one partition (bf16) so step t is a row slice
                tsnb = lp.tile([T, CW], bf16, tag="tsnb")
                nc.vector.tensor_copy(out=tsnb[:], in_=tsn[:])
                tsf = lp.tile([1, T, CW], bf16, tag="tsf")
                nc.sync.dma_start(out=tsf[:], in_=tsnb[:])

                # ---------- LSTM state (f32) + bf16 matmul views ----------
                h0 = lp.tile([D, CW], f32, tag="h0")
                nc.vector.memset(h0[:], 0.0)
                c0 = lp.tile([D, CW], f32, tag="c0")
                nc.vector.memset(c0[:], 0.0)
                h1 = lp.tile([D, CW], f32, tag="h1")
                nc.vector.memset(h1[:], 0.0)
                c1 = lp.tile([D, CW], f32, tag="c1")
                nc.vector.memset(c1[:], 0.0)
                # h0 handoff to layer 1 is double-buffered so layer-0 step
                # t+1 can run before layer-1 step t (pipelined layers)
                h0b0 = lp.tile([D, CW], bf16, tag="h0b0")
                nc.vector.memset(h0b0[:], 0.0)
                h0b1 = lp.tile([D, CW], bf16, tag="h0b1")
                nc.vector.memset(h0b1[:], 0.0)
                h0bs = [h0b0, h0b1]
                h1b = lp.tile([D, CW], bf16, tag="h1b")
                nc.vector.memset(h1b[:], 0.0)

                def lstm_step(hb, c_st, w_hh, w_ih, x_ap, b_col, h_out, hb_out):
                    # gates: pg[:, k, :] = Whh[:,k] @ hb + Wih[:,k] @ x
                    pg = pR1.tile([D, 4, CW], f32, tag="ls", bufs=2)
                    for k in range(4):
                        sl = slice(D * k, D * (k + 1))
                        nc.tensor.matmul(out=pg[:, k, :], lhsT=w_hh[:, sl],
                                         rhs=hb[:], start=True, stop=False)
                        nc.tensor.matmul(out=pg[:, k, :], lhsT=w_ih[:, sl],
                                         rhs=x_ap, start=False, stop=True)
                    # biases folded into the activations (per-partition)
                    si = lp.tile([D, CW], f32, tag="si")
                    nc.scalar.activation(out=si[:], in_=pg[:, 0, :],
                                         func=AF.Sigmoid, bias=b_col[:, 0:1])
                    sf = lp.tile([D, CW], f32, tag="sf")
                    nc.scalar.activation(out=sf[:], in_=pg[:, 1, :],
                                         func=AF.Sigmoid, bias=b_col[:, 1:2])
                    tg = lp.tile([D, CW], f32, tag="tg")
                    nc.scalar.activation(out=tg[:], in_=pg[:, 2, :],
                                         func=AF.Tanh, bias=b_col[:, 2:3])
                    so = lp.tile([D, CW], f32, tag="so")
                    nc.scalar.activation(out=so[:], in_=pg[:, 3, :],
                                         func=AF.Sigmoid, bias=b_col[:, 3:4])
                    t1 = lp.tile([D, CW], f32, tag="t1")
                    nc.vector.tensor_mul(out=t1[:], in0=si[:], in1=tg[:])
                    nc.vector.tensor_mul(out=c_st[:], in0=sf[:], in1=c_st[:])
                    nc.vector.tensor_add(out=c_st[:], in0=c_st[:], in1=t1[:])
                    tcs = lp.tile([D, CW], f32, tag="tcs")
                    nc.scalar.activation(out=tcs[:], in_=c_st[:], func=AF.Tanh)
                    nc.vector.tensor_mul(out=h_out[:], in0=so[:], in1=tcs[:])
                    nc.vector.tensor_copy(out=hb_out[:], in_=h_out[:])

                def l0_step(t):
                    lstm_step(h0bs[(t + 1) % 2], c0, whh0_s, wih0r_s,
                              tsf[:, t, :], b0c_s, h0, h0bs[t % 2])

                def l1_step(t):
                    lstm_step(h1b, c1, whh1_s, wih1t_s,
                              h0bs[t % 2][:], b1c_s, h1, h1b)

                # pipelined order: l0(t) ahead of l1(t-1) so the critical
                # path is ~16 step latencies instead of 30
                lstm_units = [lambda: l0_step(0)]
                for t in range(1, T):
                    lstm_units.append(lambda t=t: l0_step(t))
                    lstm_units.append(lambda t=t: l1_step(t - 1))
                lstm_units.append(lambda: l1_step(T - 1))

                ag_in = dpool.tile([CW, D], f32, tag="ag_in")
                ag_out = dpool.tile([CP, D], f32, tag="ag_out")
                comp0r = rpool.tile([128, 5, D], f32, tag="comp0r")
                comp0rb = rpool.tile([128, 5, D], bf16, tag="comp0rb")

                def lstm_finish():
                    # comp_my = relu(fc(h1)) + emb -> rows [CW, 64] -> AllGather
                    pfc = pR1.tile([D, CW], f32, tag="ls", bufs=2)
                    nc.tensor.matmul(out=pfc[:], lhsT=fcw_s[:], rhs=h1[:],
                                     start=True, stop=False)
                    nc.tensor.matmul(out=pfc[:], lhsT=fcb_s[:],
                                     rhs=ones_r[:, :CW], start=False, stop=True)
                    cmT = lp.tile([D, CW], f32, tag="cmT")
                    nc.scalar.activation(out=cmT[:], in_=pfc[:], func=AF.Relu)
                    nc.vector.tensor_add(out=cmT[:], in0=cmT[:], in1=cemb_s[:])
                    pcr = pR1.tile([CW, D], f32, tag="ls", bufs=2)
                    nc.tensor.transpose(out=pcr[:], in_=cmT[:],
                                        identity=ident[:D, :D])
                    cmr = lp.tile([CW, D], f32, tag="cmr")
                    nc.vector.tensor_copy(out=cmr[:], in_=pcr[:])
                    nc.sync.dma_start(out=ag_in[:], in_=cmr[:])
                    nc.gpsimd.collective_compute(
                        "AllGather", OP.bypass,
                        replica_groups=[list(range(NCORES))],
                        ins=[ag_in.opt()], outs=[ag_out.opt()])
                    # comp0 rows for the stream loop, issued right away so
                    # loop B can start the moment the AllGather lands
                    nc.sync.dma_start(
                        out=comp0r[:],
                        in_=ag_out[:].rearrange("(q p) f -> p q f", p=128))
                    nc.vector.tensor_copy(out=comp0rb[:], in_=comp0r[:])

                lstm_units.append(lstm_finish)

                # ---------- projection loop (+LSTM interleave, 2/sc) ----
                ui = 0
                for sc in range(NSC):
                    xt_t = xpool.tile([128, 4, 6, 128], bf16, tag="xq")
                    nc.sync.dma_start(out=xt_t[:], in_=xq_d[:, sc])
                    for c in range(4):
                        j = sc * 4 + c
                        psA = pR1.tile([128, D], f32, tag="a", bufs=3)
                        for q in range(6):
                            nc.tensor.matmul(out=psA[:], lhsT=xt_t[:, c, q, :],
                                             rhs=wprojr_s[:, q, :],
                                             start=(q == 0), stop=(q == 5))
                        nc.vector.tensor_copy(out=scat[:, j, 0, :], in_=psA[:])
                    for _ in range(2):
                        if ui < len(lstm_units):
                            lstm_units[ui]()
                            ui += 1
                while ui < len(lstm_units):
                    lstm_units[ui]()
                    ui += 1

            # ---------- company-update helpers (pool-parametric) --------
            def t_to_rows(srcT, tag, pool):
                # [64, 640] -> bf16 rows [128, 5, 64]
                rowsb = rpool.tile([128, 5, D], bf16, tag=tag + "b",
                                   name=tag + "b")
                for q in range(5):
                    pt = pool.tile([128, D], f32, tag="small", bufs=2,
                                   name="pt")
                    nc.tensor.transpose(out=pt[:],
                                        in_=srcT[:, 128 * q:128 * (q + 1)],
                                        identity=ident[:D, :D])
                    nc.vector.tensor_copy(out=rowsb[:, q, :], in_=pt[:])
                return rowsb

            def rows_to_t(rows_f32, tag, pool):
                # f32 rows [128, 5, 64] -> [64, 640]
                dstT = rpool.tile([D, CP], f32, tag=tag, name=tag)
                for q in range(5):
                    pt = pool.tile([D, 128], f32, tag="small", bufs=2,
                                   name="pt")
                    nc.tensor.transpose(out=pt[:], in_=rows_f32[:, q, :],
                                        identity=ident[:])
                    nc.vector.tensor_copy(out=dstT[:, 128 * q:128 * (q + 1)],
                                          in_=pt[:])
                return dstT

            def c2c_msg(rowsb, pool):
                p = pool.tile([D, CP], f32, tag="big", bufs=2, name="pc2c")
                for q in range(5):
                    for sl, sz in ((0, 512), (512, 128)):
                        nc.tensor.matmul(out=p[:, sl:sl + sz],
                                         lhsT=rowsb[:, q, :],
                                         rhs=c3t_s[:, q, sl:sl + sz],
                                         start=(q == 0), stop=(q == 4))
                return p

            def gin_update(compT, msg_ap, l, r, tag, pool):
                y = wk.tile([D, CP], bf16, tag=f"y{tag}", name=f"y{tag}",
                            bufs=1)
                nc.vector.tensor_tensor(out=y[:], in0=msg_ap, in1=compT[:],
                                        op=OP.add)
                ph = pool.tile([D, CP], f32, tag="big", bufs=2, name="ph")
                for sl, sz in ((0, 512), (512, 128)):
                    nc.tensor.matmul(out=ph[:, sl:sl + sz],
                                     lhsT=gw_s[(l, r)][:],
                                     rhs=y[:, sl:sl + sz],
                                     start=True, stop=False)
                    nc.tensor.matmul(out=ph[:, sl:sl + sz],
                                     lhsT=gb_s[(l, r)][:],
                                     rhs=ones_rb[:, sl:sl + sz],
                                     start=False, stop=True)
                o = wk.tile([D, CP], f32, tag=f"o{tag}", name=f"o{tag}",
                            bufs=1)
                av = float(a_vals[l, r])
                nc.scalar.mul(out=o[:], in_=ph[:], mul=av)
                nc.vector.tensor_max(out=o[:], in0=o[:], in1=ph[:])
                return o

            # ===== region 1.5: AR-independent company work ==============
            # (gated only on the AllGather; overlaps the loop A -> B seam)
            with tc.tile_pool(name="ps15", bufs=1, space="PSUM") as pR15:
                pcorr_ps = pR15.tile([D, CP], f32, tag="big", bufs=2)
                for sl, sz in ((0, 512), (512, 128)):
                    nc.tensor.matmul(out=pcorr_ps[:, sl:sl + sz],
                                     lhsT=projb_s[:],
                                     rhs=deg1_s[:, sl:sl + sz],
                                     start=True, stop=True)
                pcorr = rpool.tile([D, CP], f32, tag="pcorr")
                nc.vector.tensor_copy(out=pcorr[:], in_=pcorr_ps[:])
                comp0T = rows_to_t(comp0r, "comp0T", pR15)
                pc2c0 = c2c_msg(comp0rb, pR15)
                ha = gin_update(comp0T, pc2c0[:], 0, 0, "a1", pR15)

            ar_in_a = dpool.tile([128, CP], f32, tag="ar_in_a")
            ar_out_a = dpool.tile([128, CP], f32, tag="ar_out_a")
            ar_in_b = dpool.tile([128, CP], f32, tag="ar_in_b")
            ar_out_b = dpool.tile([128, CP], f32, tag="ar_out_b")

            # =========== region 2: GNN streaming loop (sw-pipelined) ====
            with tc.tile_pool(name="ps2", bufs=1, space="PSUM") as pR2:
                msg12a = pR2.tile([128, CP], f32, tag="ma", bufs=1)
                msg12b = pR2.tile([128, CP], f32, tag="mb", bufs=1)

                def flush_half(mm, ar_in, ar_out, tag):
                    m_sb = rpool.tile([128, CP], f32, tag=tag, name=tag)
                    nc.vector.tensor_copy(out=m_sb[:], in_=mm[:])
                    nc.sync.dma_start(out=ar_in[:], in_=m_sb[:])
                    nc.gpsimd.collective_compute(
                        "AllReduce", OP.add,
                        replica_groups=[list(range(NCORES))],
                        ins=[ar_in.opt()], outs=[ar_out.opt()])

                c1_tiles = {}
                xs_tiles = {}
                xtb_tiles = {}
                ps1_tiles = {}
                pst_tiles = {}

                def stage_a(j):
                    # c2s message + xs (sent0 + msg, ones column)
                    sc, c = j // 4, j % 4
                    c2_t = c2_tiles[sc]
                    psB = pR2.tile([128, D], f32, tag="bp", bufs=3)
                    for q in range(5):
                        nc.tensor.matmul(out=psB[:], lhsT=c2_t[:, c, q, :],
                                         rhs=comp0rb[:, q, :],
                                         start=(q == 0), stop=(q == 4))
                    xs_aug = wk.tile([128, D + 1], f32, tag="xsa")
                    nc.vector.tensor_tensor(out=xs_aug[:, 0:D], in0=psB[:],
                                            in1=scat[:, j, 0, :], op=OP.add)
                    nc.vector.memset(xs_aug[:, D:D + 1], 1.0)
                    xs_tiles[j] = xs_aug

                def stage_b(j):
                    psT = pR2.tile([D + 1, 128], f32, tag="t", bufs=1)
                    nc.tensor.transpose(out=psT[:], in_=xs_tiles[j][:],
                                        identity=ident[:])
                    xTb = wk.tile([D + 1, 128], bf16, tag="xtb")
                    nc.vector.tensor_copy(out=xTb[:], in_=psT[:])
                    xtb_tiles[j] = xTb
                    del xs_tiles[j]

                def stage_c(j):
                    # sent1 = prelu(xs @ W02.T + b02')
                    ps1 = pR2.tile([128, D], f32, tag="bp", bufs=3)
                    nc.tensor.matmul(out=ps1[:], lhsT=xtb_tiles[j][:],
                                     rhs=gw02aug_s[:], start=True, stop=True)
                    pr = wk.tile([128, D], f32, tag="pr")
                    nc.vector.tensor_scalar_mul(pr[:], ps1[:], a02)
                    nc.vector.tensor_max(out=scat[:, j, 1, :],
                                         in0=pr[:], in1=ps1[:])
                    del xtb_tiles[j]

                def stage_d(j):
                    # fused s2c for both layers
                    sc, c = j // 4, j % 4
                    c1_t = c1_tiles[sc]
                    mm = msg12a if j < AR_SPLIT_J else msg12b
                    j_first = 0 if j < AR_SPLIT_J else AR_SPLIT_J
                    j_last = AR_SPLIT_J - 1 if j < AR_SPLIT_J else NCH - 1
                    for sl, sz in ((0, 512), (512, 128)):
                        nc.tensor.matmul(out=mm[:, sl:sl + sz],
                                         lhsT=scat[:, j, :, :],
                                         rhs=c1_t[:, c, sl:sl + sz],
                                         start=(j == j_first),
                                         stop=(j == j_last))
                    if j == AR_SPLIT_J - 1:
                        flush_half(msg12a, ar_in_a, ar_out_a, "m12a")

                c2_tiles = {}
                for sc in range(NSC):
                    c1_t = c1pool.tile([128, 4, CP], f8, tag="c1")
                    nc.sync.dma_start(out=c1_t[:], in_=c1t_d[:, sc])
                    c1_tiles[sc] = c1_t
                    c2_t = c2pool.tile([128, 4, 5, 128], f8, tag="c2")
                    nc.sync.dma_start(out=c2_t[:], in_=c2t_d[:, sc])
                    c2_tiles[sc] = c2_t
                    for c in range(4):
                        j = sc * 4 + c
                        stage_a(j)
                        if j >= 1:
                            stage_b(j - 1)
                        if j >= 2:
                            stage_c(j - 2)
                        if j >= 3:
                            stage_d(j - 3)
                stage_b(NCH - 1)
                stage_c(NCH - 2)
                stage_c(NCH - 1)
                for j in (NCH - 3, NCH - 2, NCH - 1):
                    stage_d(j)
                flush_half(msg12b, ar_in_b, ar_out_b, "m12b")

            m1a = rpool.tile([D, CP], f32, tag="m1a")
            nc.sync.dma_start(out=m1a[:], in_=ar_out_a[:D, :])
            m1b = rpool.tile([D, CP], f32, tag="m1b")
            nc.sync.dma_start(out=m1b[:], in_=ar_out_b[:D, :])
            m2a = rpool.tile([D, CP], f32, tag="m2a")
            nc.sync.dma_start(out=m2a[:], in_=ar_out_a[D:, :])
            m2b = rpool.tile([D, CP], f32, tag="m2b")
            nc.sync.dma_start(out=m2b[:], in_=ar_out_b[D:, :])

            # =========== region 3: AR-dependent company updates =========
            with tc.tile_pool(name="ps3", bufs=1, space="PSUM") as pR3:
                msgr1 = rpool.tile([D, CP], f32, tag="msgr1")
                nc.vector.tensor_add(out=msgr1[:], in0=m1a[:], in1=m1b[:])
                nc.vector.tensor_add(out=msgr1[:], in0=msgr1[:], in1=pcorr[:])
                msgr2 = rpool.tile([D, CP], f32, tag="msgr2")
                nc.vector.tensor_add(out=msgr2[:], in0=m2a[:], in1=m2b[:])

                # layer 1 (ha precomputed in region 1.5)
                hb = gin_update(comp0T, msgr1[:], 0, 1, "b1", pR3)
                comp1T = rpool.tile([D, CP], f32, tag="comp1T")
                nc.vector.tensor_add(out=comp1T[:], in0=ha[:], in1=hb[:])
                comp1rb = t_to_rows(comp1T, "comp1r", pR3)
                # layer 2
                pc2c1 = c2c_msg(comp1rb, pR3)
                ha2 = gin_update(comp1T, pc2c1[:], 1, 0, "a2", pR3)
                hb2 = gin_update(comp1T, msgr2[:], 1, 1, "b2", pR3)
                comp2T = rpool.tile([D, CP], f32, tag="comp2T")
                nc.vector.tensor_add(out=comp2T[:], in0=ha2[:], in1=hb2[:])

                # classifier (f32)
                pcls = pR3.tile([2, CP], f32, tag="big", bufs=2)
                for sl, sz in ((0, 512), (512, 128)):
                    nc.tensor.matmul(out=pcls[:, sl:sl + sz], lhsT=clsw_s[:],
                                     rhs=comp2T[:, sl:sl + sz],
                                     start=True, stop=False)
                    nc.tensor.matmul(out=pcls[:, sl:sl + sz], lhsT=clsb_s[:],
                                     rhs=ones_r[:, sl:sl + sz],
                                     start=False, stop=True)
                outs = wk.tile([2, CP], f32, tag="outs", bufs=1)
                nc.vector.tensor_copy(out=outs[:], in_=pcls[:])
                nc.sync.dma_start(out=out_d[:], in_=outs[:])

    nc.compile()
    return nc


_CACHE = {}


def _get_program(a_vals):
    key = np.asarray(a_vals, np.float64).tobytes()
    if key not in _CACHE:
        _CACHE[key] = _build(a_vals)
    return _CACHE[key]


def _prep_inputs(inp):
    """Host-side sharding, layout swizzles, count-matrix construction."""
    sx = np.asarray(inp["sentence_x"], np.float32)
    cts = np.asarray(inp["company_ts"], np.float32)
    cids = np.asarray(inp["company_ids"]).astype(np.int64)
    emb = np.asarray(inp["comp_emb"], np.float32)

    tsq = np.zeros((CP, T), np.float32)
    tsq[:NC] = cts[:, :, F - 2]
    cembT = np.zeros((D, CP), np.float32)
    cembT[:, :NC] = emb[cids].T

    s2c_s = np.asarray(inp["ei_s2c_src"]).astype(np.int64)
    s2c_d = np.asarray(inp["ei_s2c_dst"]).astype(np.int64)
    c2s_s = np.asarray(inp["ei_c2s_src"]).astype(np.int64)
    c2s_d = np.asarray(inp["ei_c2s_dst"]).astype(np.int64)
    c2c_s = np.asarray(inp["ei_c2c_src"]).astype(np.int64)
    c2c_d = np.asarray(inp["ei_c2c_dst"]).astype(np.int64)

    c3t = np.bincount(c2c_s * CP + c2c_d, minlength=CP * CP).reshape(
        CP, CP).astype(np.float32)
    # [640, 640] -> [128, 5, 640] (partition = src % 128, block = src // 128)
    c3t8 = np.ascontiguousarray(
        c3t.reshape(5, 128, CP).transpose(1, 0, 2)).astype(F8)

    deg1 = np.bincount(s2c_d, minlength=CP).astype(np.float32).reshape(1, CP)

    core1 = s2c_s // SS
    loc1 = s2c_s - core1 * SS
    core2 = c2s_d // SS
    loc2 = c2s_d - core2 * SS

    per_core = []
    for k in range(NCORES):
        m1 = core1 == k
        cnt1 = np.bincount(loc1[m1] * CP + s2c_d[m1],
                           minlength=SP * CP).reshape(SP, CP)
        c1t = np.ascontiguousarray(
            cnt1.reshape(NSC, 4, 128, CP).transpose(2, 0, 1, 3)).astype(F8)
        del cnt1
        m2 = core2 == k
        cnt2 = np.bincount(c2s_s[m2] * SP + loc2[m2],
                           minlength=CP * SP).reshape(CP, SP)
        c2t = np.ascontiguousarray(
            cnt2.reshape(5, 128, NSC, 4, 128).transpose(1, 2, 3, 0, 4)
        ).astype(F8)
        del cnt2
        xk = np.zeros((SP, A), np.float32)
        xk[:SS] = sx[SS * k:SS * (k + 1)]
        xq = np.ascontiguousarray(
            xk.reshape(NSC, 4, 128, 6, 128).transpose(4, 0, 1, 3, 2)
        ).astype(BF)
        del xk
        per_core.append({
            "xq": xq, "c1t": c1t, "c2t": c2t,
            "tsmy": np.ascontiguousarray(tsq[CW * k:CW * (k + 1)]),
            "cembmy": np.ascontiguousarray(cembT[:, CW * k:CW * (k + 1)]),
        })

    gin_W = np.asarray(inp["gin_W"], np.float32)
    gin_b = np.asarray(inp["gin_b"], np.float32)
    proj_W = np.asarray(inp["proj_W"], np.float32)
    proj_b = np.asarray(inp["proj_b"], np.float32)
    # b02' = b02 + W02 @ proj_b (proj_b folded out of the sentence stream)
    b02p = gin_b[0, 2] + gin_W[0, 2] @ proj_b
    gw02aug = np.concatenate([gin_W[0, 2].T, b02p.reshape(1, D)],
                             axis=0).astype(BF)
    wprojr = np.ascontiguousarray(
        proj_W.T.reshape(6, 128, D).transpose(1, 0, 2)).astype(BF)

    b0 = (np.asarray(inp["lstm_bih0"], np.float32)
          + np.asarray(inp["lstm_bhh0"], np.float32))
    b1 = (np.asarray(inp["lstm_bih1"], np.float32)
          + np.asarray(inp["lstm_bhh1"], np.float32))

    lw = {
        "c3t8": c3t8, "tsq": tsq,
        "bn_g": np.asarray(inp["bn_gamma"], np.float32).reshape(T, 1),
        "bn_b": np.asarray(inp["bn_beta"], np.float32).reshape(T, 1),
        "wih0r": np.asarray(inp["lstm_Wih0"], np.float32)[:, 0].reshape(
            1, 4 * D).astype(BF),
        "b0col": np.ascontiguousarray(b0.reshape(4, D).T),
        "whh0": np.ascontiguousarray(
            np.asarray(inp["lstm_Whh0"], np.float32).T).astype(BF),
        "wih1t": np.ascontiguousarray(
            np.asarray(inp["lstm_Wih1"], np.float32).T).astype(BF),
        "b1col": np.ascontiguousarray(b1.reshape(4, D).T),
        "whh1": np.ascontiguousarray(
            np.asarray(inp["lstm_Whh1"], np.float32).T).astype(BF),
        "fcw": np.ascontiguousarray(np.asarray(inp["fc_W"], np.float32).T),
        "fcb": np.asarray(inp["fc_b"], np.float32).reshape(1, D),
        "wprojr": wprojr,
        "gw02aug": gw02aug,
        "clsw": np.ascontiguousarray(np.asarray(inp["cls_W"], np.float32).T),
        "clsb": np.asarray(inp["cls_b"], np.float32).reshape(1, 2),
        "deg1row": deg1,
        "projbcol": proj_b.reshape(1, D),
        "ones128": np.ones((128, 1), np.float32),
        "epsd": np.full((T, 1), 1e-5, np.float32),
        "onesrow": np.ones((1, CP), np.float32),
        "onesrowb": np.ones((1, CP), BF),
    }
    for l in range(2):
        for r in range(2):
            lw[f"gw{l}{r}"] = np.ascontiguousarray(gin_W[l, r].T).astype(BF)
            lw[f"gb{l}{r}"] = gin_b[l, r].reshape(1, D).astype(BF)

    in_maps = [{**per_core[k], **lw} for k in range(NCORES)]
    return in_maps


def kernel(**inputs):
    inp = {k: np.asarray(v) for k, v in inputs.items()}
    a_vals = np.asarray(inp["gin_a"], np.float32)
    nc = _get_program(a_vals)
    in_maps = _prep_inputs(inp)
    res = run_bass_kernel_spmd(nc, in_maps, list(range(NCORES)))
    out = np.asarray(res.results[0]["out"])  # [2, CP]
    return np.ascontiguousarray(out.T[:NC]).astype(np.float32)


if __name__ == "__main__":
    # quick self-test against the reference
    sys.path.insert(0, "/root/problem")
    import reference

    inputs = {k: np.asarray(v) for k, v in reference.setup_inputs().items()}
    expected = np.asarray(reference.reference(**reference.setup_inputs()))
    got = kernel(**inputs)
    err = np.abs(got - expected).max() / (np.abs(expected).max() + 1e-30)
    print("Relative error:", err)


# revision 18
# speedup vs baseline: 1.2460x; 1.0406x over previous
"""NeuroStockBloom GNN kernel for 8 Trainium2 NeuronCores.

Strategy
--------
Data-parallel over the 200000 sentence nodes: 25000 sentences per core
(padded to 25088 = 49*4*128). Company side (617 nodes, padded to 640) is
replicated.

Graph aggregations are dense matmuls against count matrices built on the
host from the edge lists (small integers, exact in fp8 e4m3):
  - s2c: msg12[g, c] += scat[s, g]^T cnt1[s, c]  (both GIN layers fused
    in one [128, 640] PSUM accumulation; one AllReduce, split in two
    halves so the first overlaps the second half of the stream)
  - c2s: msg_s[s, f]  = cnt2[c, s]^T comp0[c, f]
  - c2c: resident [640, 640] fp8 count matrix.

Counts stream as fp8 (exact); all value tensors are bf16 (fp8 values
cost ~3% relative error, which random-sign sums do NOT average away).
Host-swizzled layouts make each DMA one large contiguous transfer.

Bias folding: proj_b is folded out of the sentence stream; its effect on
the layer-1 s2c message is restored with a rank-1 deg1 x proj_b term on
the company side, and on the sentence update via b02' = b02 + W02 @
proj_b, which rides row 64 of the augmented transpose (ones column).
LSTM biases are folded into the scalar-engine activations (per-partition
bias operand), removing 4 matmuls per step; LSTM weight matmuls run in
bf16 (state stays f32).

The GNN stream loop is software-pipelined (stages lagged by 1-3 chunks)
so every PE instruction's DVE/scalar-produced inputs are ready chunks in
advance - the in-order PE queue never stalls on the per-chunk
vector/scalar round trip, which also keeps the PE HAM-warm.
"""

import sys

import numpy as np
import ml_dtypes

import concourse.bass as bass
import concourse.bacc as bacc
import concourse.mybir as mybir
import concourse.tile as tile
from concourse.bass_utils import run_bass_kernel_spmd
from concourse.masks import make_identity

NCORES = 8
NC, S, T, F, D, A = 617, 200000, 15, 5, 64, 768
SS = S // NCORES          # 25000 sentences per core
SP = 25088                # padded (49 superchunks of 4 chunks of 128)
NSC = 49                  # superchunks per core
NCH = 196                 # chunks per core
CP = 640                  # padded companies (5 blocks of 128)
CW = CP // NCORES         # 80 companies per core for the LSTM branch
f32 = mybir.dt.float32
bf16 = mybir.dt.bfloat16
f8 = mybir.dt.float8e4
BF = ml_dtypes.bfloat16
F8 = ml_dtypes.float8_e4m3
AF = mybir.ActivationFunctionType
OP = mybir.AluOpType

AR_SPLIT_J = 100          # chunks 0..99 -> AllReduce #1, 100..195 -> #2


def _build(a_vals):
    """Build the SPMD bass program. a_vals: [2, 3] PReLU slopes."""
    a_vals = np.asarray(a_vals, np.float64)
    assert np.all(a_vals >= 0) and np.all(a_vals <= 1), "prelu-as-max needs 0<=a<=1"

    nc = bacc.Bacc("TRN2", target_bir_lowering=False, debug=False,
                   num_devices=NCORES)

    # ---- per-core inputs ----
    xq_d = nc.dram_tensor("xq", [128, NSC, 4, 6, 128], bf16, kind="ExternalInput")
    c1t_d = nc.dram_tensor("c1t", [128, NSC, 4, CP], f8, kind="ExternalInput")
    c2t_d = nc.dram_tensor("c2t", [128, NSC, 4, 5, 128], f8, kind="ExternalInput")
    tsmy = nc.dram_tensor("tsmy", [CW, T], f32, kind="ExternalInput")
    cembmy = nc.dram_tensor("cembmy", [D, CW], f32, kind="ExternalInput")
    # ---- replicated inputs ----
    c3t_d = nc.dram_tensor("c3t8", [128, 5, CP], f8, kind="ExternalInput")
    tsq = nc.dram_tensor("tsq", [CP, T], f32, kind="ExternalInput")
    bn_g = nc.dram_tensor("bn_g", [T, 1], f32, kind="ExternalInput")
    bn_b = nc.dram_tensor("bn_b", [T, 1], f32, kind="ExternalInput")
    wih0r = nc.dram_tensor("wih0r", [1, 4 * D], bf16, kind="ExternalInput")
    b0col = nc.dram_tensor("b0col", [D, 4], f32, kind="ExternalInput")
    whh0 = nc.dram_tensor("whh0", [D, 4 * D], bf16, kind="ExternalInput")
    wih1t = nc.dram_tensor("wih1t", [D, 4 * D], bf16, kind="ExternalInput")
    b1col = nc.dram_tensor("b1col", [D, 4], f32, kind="ExternalInput")
    whh1 = nc.dram_tensor("whh1", [D, 4 * D], bf16, kind="ExternalInput")
    fcw = nc.dram_tensor("fcw", [D, D], f32, kind="ExternalInput")
    fcb = nc.dram_tensor("fcb", [1, D], f32, kind="ExternalInput")
    wprojr = nc.dram_tensor("wprojr", [128, 6, D], bf16, kind="ExternalInput")
    gw02aug = nc.dram_tensor("gw02aug", [D + 1, D], bf16, kind="ExternalInput")
    gw = {}
    gb = {}
    for l in range(2):
        for r in range(2):
            gw[(l, r)] = nc.dram_tensor(f"gw{l}{r}", [D, D], bf16,
                                        kind="ExternalInput")
            gb[(l, r)] = nc.dram_tensor(f"gb{l}{r}", [1, D], bf16,
                                        kind="ExternalInput")
    clsw = nc.dram_tensor("clsw", [D, 2], f32, kind="ExternalInput")
    clsb = nc.dram_tensor("clsb", [1, 2], f32, kind="ExternalInput")
    deg1row = nc.dram_tensor("deg1row", [1, CP], f32, kind="ExternalInput")
    projbcol = nc.dram_tensor("projbcol", [1, D], f32, kind="ExternalInput")
    ones128 = nc.dram_tensor("ones128", [128, 1], f32, kind="ExternalInput")
    epsd = nc.dram_tensor("epsd", [T, 1], f32, kind="ExternalInput")
    onesrow = nc.dram_tensor("onesrow", [1, CP], f32, kind="ExternalInput")
    onesrowb = nc.dram_tensor("onesrowb", [1, CP], bf16, kind="ExternalInput")
    out_d = nc.dram_tensor("out", [2, CP], f32, kind="ExternalOutput")

    a02 = float(a_vals[0, 2])

    with tile.TileContext(nc) as tc:
        with (
            tc.tile_pool(name="const", bufs=1) as cpool,
            tc.tile_pool(name="res", bufs=1) as rpool,
            tc.tile_pool(name="xs", bufs=5) as xpool,
            tc.tile_pool(name="c1s", bufs=5) as c1pool,
            tc.tile_pool(name="c2s", bufs=5) as c2pool,
            tc.tile_pool(name="wk", bufs=4) as wk,
            tc.tile_pool(name="lstm", bufs=1) as lp,
            tc.tile_pool(name="dram", bufs=1, space="DRAM") as dpool,
        ):
            # ---------- constants ----------
            ident = cpool.tile([128, 128], f32)
            make_identity(nc, ident[:])
            ones_c = cpool.tile([128, 1], f32, tag="onesc")
            nc.sync.dma_start(out=ones_c[:], in_=ones128[:])
            ones_r = cpool.tile([1, CP], f32, tag="onesr")
            nc.sync.dma_start(out=ones_r[:], in_=onesrow[:])
            ones_rb = cpool.tile([1, CP], bf16, tag="onesrb")
            nc.sync.dma_start(out=ones_rb[:], in_=onesrowb[:])

            def load_const(name, dram, shape, dtype=f32):
                t = cpool.tile(shape, dtype, tag=name, name=name)
                nc.sync.dma_start(out=t[:], in_=dram[:])
                return t

            wih0r_s = load_const("wih0r", wih0r, [1, 4 * D], bf16)
            b0c_s = load_const("b0col", b0col, [D, 4])
            whh0_s = load_const("whh0", whh0, [D, 4 * D], bf16)
            wih1t_s = load_const("wih1t", wih1t, [D, 4 * D], bf16)
            b1c_s = load_const("b1col", b1col, [D, 4])
            whh1_s = load_const("whh1", whh1, [D, 4 * D], bf16)
            fcw_s = load_const("fcw", fcw, [D, D])
            fcb_s = load_const("fcb", fcb, [1, D])
            wprojr_s = load_const("wprojr", wprojr, [128, 6, D], bf16)
            gw02aug_s = load_const("gw02aug", gw02aug, [D + 1, D], bf16)
            gw_s = {k: load_const(f"gw{k[0]}{k[1]}", v, [D, D], bf16)
                    for k, v in gw.items()}
            gb_s = {k: load_const(f"gb{k[0]}{k[1]}", v, [1, D], bf16)
                    for k, v in gb.items()}
            clsw_s = load_const("clsw", clsw, [D, 2])
            clsb_s = load_const("clsb", clsb, [1, 2])
            deg1_s = load_const("deg1row", deg1row, [1, CP])
            projb_s = load_const("projbcol", projbcol, [1, D])
            eps_s = load_const("epsd", epsd, [T, 1])
            bn_g_s = load_const("bn_g", bn_g, [T, 1])
            bn_b_s = load_const("bn_b", bn_b, [T, 1])
            cemb_s = load_const("cembmy", cembmy, [D, CW])
            c3t_s = load_const("c3t8", c3t_d, [128, 5, CP], f8)

            # persistent sentence-feature store: [sent0 | sent1] rows, bf16
            scat = rpool.tile([128, NCH, 2, D], bf16, tag="scat")

            # =========== region 1: BN stats + LSTM + projection loop ====
            with tc.tile_pool(name="ps1", bufs=1, space="PSUM") as pR1:
                # ---------- HAM warmup: ~4.5us of back-to-back matmuls --
                # unthrottles the PE clock to 2.4 GHz before the stream;
                # the loops' sub-3.4us gaps then never re-throttle it
                dummy = wk.tile([128, 512], bf16, tag="dummy", bufs=1)
                nc.vector.memset(dummy[:], 0.0)
                pwarm = pR1.tile([D, 512], f32, tag="warm", bufs=1)
                for _ in range(10):
                    nc.tensor.matmul(out=pwarm[:], lhsT=wprojr_s[:, 0, :],
                                     rhs=dummy[:], start=True, stop=True)

                # ---------- BatchNorm stats (replicated, tiny) ----------
                tsch = wk.tile([128, 5, T], f32, tag="tsch", bufs=1)
                nc.sync.dma_start(
                    out=tsch[:], in_=tsq.ap().rearrange("(q p) t -> p q t", p=128))
                sq = wk.tile([128, 5, T], f32, tag="tssq", bufs=1)
                nc.vector.tensor_mul(out=sq[:], in0=tsch[:], in1=tsch[:])
                psums = pR1.tile([T, 2], f32, tag="ls", bufs=2)
                for q in range(5):
                    nc.tensor.matmul(out=psums[:, 0:1], lhsT=tsch[:, q, :],
                                     rhs=ones_c[:], start=(q == 0), stop=(q == 4))
                for q in range(5):
                    nc.tensor.matmul(out=psums[:, 1:2], lhsT=sq[:, q, :],
                                     rhs=ones_c[:], start=(q == 0), stop=(q == 4))
                mean = wk.tile([T, 1], f32, tag="mean", bufs=1)
                nc.scalar.mul(out=mean[:], in_=psums[:, 0:1], mul=1.0 / NC)
                msq = wk.tile([T, 1], f32, tag="msq", bufs=1)
                nc.vector.tensor_mul(out=msq[:], in0=mean[:], in1=mean[:])
                var = wk.tile([T, 1], f32, tag="var", bufs=1)
                nc.scalar.mul(out=var[:], in_=psums[:, 1:2], mul=1.0 / NC)
                nc.vector.tensor_sub(out=var[:], in0=var[:], in1=msq[:])
                nc.vector.tensor_add(out=var[:], in0=var[:], in1=eps_s[:])
                sd = wk.tile([T, 1], f32, tag="sd", bufs=1)
                nc.scalar.activation(out=sd[:], in_=var[:], func=AF.Sqrt)
                inv = wk.tile([T, 1], f32, tag="inv", bufs=1)
                nc.vector.reciprocal(out=inv[:], in_=sd[:])
                scale = wk.tile([T, 1], f32, tag="scale", bufs=1)
                nc.vector.tensor_mul(out=scale[:], in0=bn_g_s[:], in1=inv[:])
                mscaled = wk.tile([T, 1], f32, tag="mscaled", bufs=1)
                nc.vector.tensor_mul(out=mscaled[:], in0=mean[:], in1=scale[:])
                shift = wk.tile([T, 1], f32, tag="shift", bufs=1)
                nc.vector.tensor_sub(out=shift[:], in0=bn_b_s[:], in1=mscaled[:])

                # ---------- normalize this core's LSTM slice ----------
                tsmy_s = wk.tile([CW, T], f32, tag="tsmy", bufs=1)
                nc.sync.dma_start(out=tsmy_s[:], in_=tsmy[:])
                ptsT = pR1.tile([T, CW], f32, tag="ls", bufs=2)
                nc.tensor.transpose(out=ptsT[:], in_=tsmy_s[:],
                                    identity=ident[:CW, :CW])
                tsn = lp.tile([T, CW], f32, tag="tsn")
                nc.vector.tensor_tensor(out=tsn[:], in0=ptsT[:],
                                        in1=scale[:].to_broadcast([T, CW]),
                                        op=OP.mult)
                nc.vector.tensor_tensor(out=tsn[:], in0=tsn[:],
                                        in1=shift[:].to_broadcast([T, CW]),
                                        op=OP.add)
                # flatten to one partition (bf16) so step t is a row slice
                tsnb = lp.tile([T, CW], bf16, tag="tsnb")
                nc.vector.tensor_copy(out=tsnb[:], in_=tsn[:])
                tsf = lp.tile([1, T, CW], bf16, tag="tsf")
                nc.sync.dma_start(out=tsf[:], in_=tsnb[:])

                # ---------- LSTM state (f32) + bf16 matmul views ----------
                h0 = lp.tile([D, CW], f32, tag="h0")
                nc.vector.memset(h0[:], 0.0)
                c0 = lp.tile([D, CW], f32, tag="c0")
                nc.vector.memset(c0[:], 0.0)
                h1 = lp.tile([D, CW], f32, tag="h1")
                nc.vector.memset(h1[:], 0.0)
                c1 = lp.tile([D, CW], f32, tag="c1")
                nc.vector.memset(c1[:], 0.0)
                # h0 handoff to layer 1 is double-buffered so layer-0 step
                # t+1 can run before layer-1 step t (pipelined layers)
                h0b0 = lp.tile([D, CW], bf16, tag="h0b0")
                nc.vector.memset(h0b0[:], 0.0)
                h0b1 = lp.tile([D, CW], bf16, tag="h0b1")
                nc.vector.memset(h0b1[:], 0.0)
                h0bs = [h0b0, h0b1]
                h1b = lp.tile([D, CW], bf16, tag="h1b")
                nc.vector.memset(h1b[:], 0.0)

                def lstm_step(hb, c_st, w_hh, w_ih, x_ap, b_col, h_out, hb_out):
                    # gates: pg[:, k, :] = Whh[:,k] @ hb + Wih[:,k] @ x
                    pg = pR1.tile([D, 4, CW], f32, tag="ls", bufs=2)
                    for k in range(4):
                        sl = slice(D * k, D * (k + 1))
                        nc.tensor.matmul(out=pg[:, k, :], lhsT=w_hh[:, sl],
                                         rhs=hb[:], start=True, stop=False)
                        nc.tensor.matmul(out=pg[:, k, :], lhsT=w_ih[:, sl],
                                         rhs=x_ap, start=False, stop=True)
                    # biases folded into the activations (per-partition)
                    si = lp.tile([D, CW], f32, tag="si")
                    nc.scalar.activation(out=si[:], in_=pg[:, 0, :],
                                         func=AF.Sigmoid, bias=b_col[:, 0:1])
                    sf = lp.tile([D, CW], f32, tag="sf")
                    nc.scalar.activation(out=sf[:], in_=pg[:, 1, :],
                                         func=AF.Sigmoid, bias=b_col[:, 1:2])
                    tg = lp.tile([D, CW], f32, tag="tg")
                    nc.scalar.activation(out=tg[:], in_=pg[:, 2, :],
                                         func=AF.Tanh, bias=b_col[:, 2:3])
                    so = lp.tile([D, CW], f32, tag="so")
                    nc.scalar.activation(out=so[:], in_=pg[:, 3, :],
                                         func=AF.Sigmoid, bias=b_col[:, 3:4])
                    t1 = lp.tile([D, CW], f32, tag="t1")
                    nc.vector.tensor_mul(out=t1[:], in0=si[:], in1=tg[:])
                    nc.vector.tensor_mul(out=c_st[:], in0=sf[:], in1=c_st[:])
                    nc.vector.tensor_add(out=c_st[:], in0=c_st[:], in1=t1[:])
                    tcs = lp.tile([D, CW], f32, tag="tcs")
                    nc.scalar.activation(out=tcs[:], in_=c_st[:], func=AF.Tanh)
                    nc.vector.tensor_mul(out=h_out[:], in0=so[:], in1=tcs[:])
                    nc.vector.tensor_copy(out=hb_out[:], in_=h_out[:])

                def l0_step(t):
                    lstm_step(h0bs[(t + 1) % 2], c0, whh0_s, wih0r_s,
                              tsf[:, t, :], b0c_s, h0, h0bs[t % 2])

                def l1_step(t):
                    lstm_step(h1b, c1, whh1_s, wih1t_s,
                              h0bs[t % 2][:], b1c_s, h1, h1b)

                # pipelined order: l0(t) ahead of l1(t-1) so the critical
                # path is ~16 step latencies instead of 30
                lstm_units = [lambda: l0_step(0)]
                for t in range(1, T):
                    lstm_units.append(lambda t=t: l0_step(t))
                    lstm_units.append(lambda t=t: l1_step(t - 1))
                lstm_units.append(lambda: l1_step(T - 1))

                ag_in = dpool.tile([CW, D], f32, tag="ag_in")
                ag_out = dpool.tile([CP, D], f32, tag="ag_out")
                comp0r = rpool.tile([128, 5, D], f32, tag="comp0r")
                comp0rb = rpool.tile([128, 5, D], bf16, tag="comp0rb")

                def lstm_finish():
                    # comp_my = relu(fc(h1)) + emb -> rows [CW, 64] -> AllGather
                    pfc = pR1.tile([D, CW], f32, tag="ls", bufs=2)
                    nc.tensor.matmul(out=pfc[:], lhsT=fcw_s[:], rhs=h1[:],
                                     start=True, stop=False)
                    nc.tensor.matmul(out=pfc[:], lhsT=fcb_s[:],
                                     rhs=ones_r[:, :CW], start=False, stop=True)
                    cmT = lp.tile([D, CW], f32, tag="cmT")
                    nc.scalar.activation(out=cmT[:], in_=pfc[:], func=AF.Relu)
                    nc.vector.tensor_add(out=cmT[:], in0=cmT[:], in1=cemb_s[:])
                    pcr = pR1.tile([CW, D], f32, tag="ls", bufs=2)
                    nc.tensor.transpose(out=pcr[:], in_=cmT[:],
                                        identity=ident[:D, :D])
                    cmr = lp.tile([CW, D], f32, tag="cmr")
                    nc.vector.tensor_copy(out=cmr[:], in_=pcr[:])
                    nc.sync.dma_start(out=ag_in[:], in_=cmr[:])
                    nc.gpsimd.collective_compute(
                        "AllGather", OP.bypass,
                        replica_groups=[list(range(NCORES))],
                        ins=[ag_in.opt()], outs=[ag_out.opt()])
                    # comp0 rows for the stream loop, issued right away so
                    # loop B can start the moment the AllGather lands
                    nc.sync.dma_start(
                        out=comp0r[:],
                        in_=ag_out[:].rearrange("(q p) f -> p q f", p=128))
                    nc.vector.tensor_copy(out=comp0rb[:], in_=comp0r[:])

                lstm_units.append(lstm_finish)

                # ---------- projection loop (+LSTM interleave, 2/sc) ----
                ui = 0
                for sc in range(NSC):
                    xt_t = xpool.tile([128, 4, 6, 128], bf16, tag="xq")
                    nc.sync.dma_start(out=xt_t[:], in_=xq_d[:, sc])
                    for c in range(4):
                        j = sc * 4 + c
                        psA = pR1.tile([128, D], f32, tag="a", bufs=3)
                        for q in range(6):
                            nc.tensor.matmul(out=psA[:], lhsT=xt_t[:, c, q, :],
                                             rhs=wprojr_s[:, q, :],
                                             start=(q == 0), stop=(q == 5))
                        nc.vector.tensor_copy(out=scat[:, j, 0, :], in_=psA[:])
                    if ui < len(lstm_units):
                        lstm_units[ui]()
                        ui += 1
                while ui < len(lstm_units):
                    lstm_units[ui]()
                    ui += 1

            # ---------- company-update helpers (pool-parametric) --------
            def t_to_rows(srcT, tag, pool):
                # [64, 640] -> bf16 rows [128, 5, 64]
                rowsb = rpool.tile([128, 5, D], bf16, tag=tag + "b",
                                   name=tag + "b")
                for q in range(5):
                    pt = pool.tile([128, D], f32, tag="small", bufs=2,
                                   name="pt")
                    nc.tensor.transpose(out=pt[:],
                                        in_=srcT[:, 128 * q:128 * (q + 1)],
                                        identity=ident[:D, :D])
                    nc.vector.tensor_copy(out=rowsb[:, q, :], in_=pt[:])
                return rowsb

            def rows_to_t(rows_f32, tag, pool):
                # f32 rows [128, 5, 64] -> [64, 640]
                dstT = rpool.tile([D, CP], f32, tag=tag, name=tag)
                for q in range(5):
                    pt = pool.tile([D, 128], f32, tag="small", bufs=2,
                                   name="pt")
                    nc.tensor.transpose(out=pt[:], in_=rows_f32[:, q, :],
                                        identity=ident[:])
                    nc.vector.tensor_copy(out=dstT[:, 128 * q:128 * (q + 1)],
                                          in_=pt[:])
                return dstT

            def c2c_msg(rowsb, pool):
                p = pool.tile([D, CP], f32, tag="big", bufs=2, name="pc2c")
                for q in range(5):
                    for sl, sz in ((0, 512), (512, 128)):
                        nc.tensor.matmul(out=p[:, sl:sl + sz],
                                         lhsT=rowsb[:, q, :],
                                         rhs=c3t_s[:, q, sl:sl + sz],
                                         start=(q == 0), stop=(q == 4))
                return p

            def gin_update(compT, msg_ap, l, r, tag, pool):
                y = wk.tile([D, CP], bf16, tag=f"y{tag}", name=f"y{tag}",
                            bufs=1)
                nc.vector.tensor_tensor(out=y[:], in0=msg_ap, in1=compT[:],
                                        op=OP.add)
                ph = pool.tile([D, CP], f32, tag="big", bufs=2, name="ph")
                for sl, sz in ((0, 512), (512, 128)):
                    nc.tensor.matmul(out=ph[:, sl:sl + sz],
                                     lhsT=gw_s[(l, r)][:],
                                     rhs=y[:, sl:sl + sz],
                                     start=True, stop=False)
                    nc.tensor.matmul(out=ph[:, sl:sl + sz],
                                     lhsT=gb_s[(l, r)][:],
                                     rhs=ones_rb[:, sl:sl + sz],
                                     start=False, stop=True)
                o = wk.tile([D, CP], f32, tag=f"o{tag}", name=f"o{tag}",
                            bufs=1)
                av = float(a_vals[l, r])
                nc.scalar.mul(out=o[:], in_=ph[:], mul=av)
                nc.vector.tensor_max(out=o[:], in0=o[:], in1=ph[:])
                return o

            # ===== region 1.5: AR-independent company work ==============
            # (gated only on the AllGather; overlaps the loop A -> B seam)
            with tc.tile_pool(name="ps15", bufs=1, space="PSUM") as pR15:
                pcorr_ps = pR15.tile([D, CP], f32, tag="big", bufs=2)
                for sl, sz in ((0, 512), (512, 128)):
                    nc.tensor.matmul(out=pcorr_ps[:, sl:sl + sz],
                                     lhsT=projb_s[:],
                                     rhs=deg1_s[:, sl:sl + sz],
                                     start=True, stop=True)
                pcorr = rpool.tile([D, CP], f32, tag="pcorr")
                nc.vector.tensor_copy(out=pcorr[:], in_=pcorr_ps[:])
                comp0T = rows_to_t(comp0r, "comp0T", pR15)
                pc2c0 = c2c_msg(comp0rb, pR15)
                ha = gin_update(comp0T, pc2c0[:], 0, 0, "a1", pR15)

            ar_in_a = dpool.tile([128, CP], bf16, tag="ar_in_a")
            ar_out_a = dpool.tile([128, CP], bf16, tag="ar_out_a")
            ar_in_b = dpool.tile([128, CP], bf16, tag="ar_in_b")
            ar_out_b = dpool.tile([128, CP], bf16, tag="ar_out_b")

            # =========== region 2: GNN streaming loop (sw-pipelined) ====
            with tc.tile_pool(name="ps2", bufs=1, space="PSUM") as pR2:
                msg12a = pR2.tile([128, CP], f32, tag="ma", bufs=1)
                msg12b = pR2.tile([128, CP], f32, tag="mb", bufs=1)

                def flush_half(mm, ar_in, ar_out, tag):
                    m_sb = rpool.tile([128, CP], bf16, tag=tag, name=tag)
                    nc.vector.tensor_copy(out=m_sb[:], in_=mm[:])
                    nc.sync.dma_start(out=ar_in[:], in_=m_sb[:])
                    nc.gpsimd.collective_compute(
                        "AllReduce", OP.add,
                        replica_groups=[list(range(NCORES))],
                        ins=[ar_in.opt()], outs=[ar_out.opt()])

                c1_tiles = {}
                xs_tiles = {}
                xtb_tiles = {}
                ps1_tiles = {}
                pst_tiles = {}

                def stage_a(j):
                    # c2s message + xs (sent0 + msg, ones column)
                    sc, c = j // 4, j % 4
                    c2_t = c2_tiles[sc]
                    psB = pR2.tile([128, D], f32, tag="bp", bufs=3)
                    for q in range(5):
                        nc.tensor.matmul(out=psB[:], lhsT=c2_t[:, c, q, :],
                                         rhs=comp0rb[:, q, :],
                                         start=(q == 0), stop=(q == 4))
                    xs_aug = wk.tile([128, D + 1], f32, tag="xsa")
                    nc.vector.tensor_tensor(out=xs_aug[:, 0:D], in0=psB[:],
                                            in1=scat[:, j, 0, :], op=OP.add)
                    nc.vector.memset(xs_aug[:, D:D + 1], 1.0)
                    xs_tiles[j] = xs_aug

                def stage_b(j):
                    psT = pR2.tile([D + 1, 128], f32, tag="t", bufs=1)
                    nc.tensor.transpose(out=psT[:], in_=xs_tiles[j][:],
                                        identity=ident[:])
                    xTb = wk.tile([D + 1, 128], bf16, tag="xtb")
                    nc.vector.tensor_copy(out=xTb[:], in_=psT[:])
                    xtb_tiles[j] = xTb
                    del xs_tiles[j]

                def stage_c(j):
                    # sent1 = prelu(xs @ W02.T + b02')
                    ps1 = pR2.tile([128, D], f32, tag="bp", bufs=3)
                    nc.tensor.matmul(out=ps1[:], lhsT=xtb_tiles[j][:],
                                     rhs=gw02aug_s[:], start=True, stop=True)
                    pr = wk.tile([128, D], f32, tag="pr")
                    nc.vector.tensor_scalar_mul(pr[:], ps1[:], a02)
                    nc.vector.tensor_max(out=scat[:, j, 1, :],
                                         in0=pr[:], in1=ps1[:])
                    del xtb_tiles[j]

                def stage_d(j):
                    # fused s2c for both layers
                    sc, c = j // 4, j % 4
                    c1_t = c1_tiles[sc]
                    mm = msg12a if j < AR_SPLIT_J else msg12b
                    j_first = 0 if j < AR_SPLIT_J else AR_SPLIT_J
                    j_last = AR_SPLIT_J - 1 if j < AR_SPLIT_J else NCH - 1
                    for sl, sz in ((0, 512), (512, 128)):
                        nc.tensor.matmul(out=mm[:, sl:sl + sz],
                                         lhsT=scat[:, j, :, :],
                                         rhs=c1_t[:, c, sl:sl + sz],
                                         start=(j == j_first),
                                         stop=(j == j_last))
                    if j == AR_SPLIT_J - 1:
                        flush_half(msg12a, ar_in_a, ar_out_a, "m12a")

                c2_tiles = {}
                for sc in range(NSC):
                    c1_t = c1pool.tile([128, 4, CP], f8, tag="c1")
                    nc.sync.dma_start(out=c1_t[:], in_=c1t_d[:, sc])
                    c1_tiles[sc] = c1_t
                    c2_t = c2pool.tile([128, 4, 5, 128], f8, tag="c2")
                    nc.sync.dma_start(out=c2_t[:], in_=c2t_d[:, sc])
                    c2_tiles[sc] = c2_t
                    for c in range(4):
                        j = sc * 4 + c
                        stage_a(j)
                        if j >= 1:
                            stage_b(j - 1)
                        if j >= 2:
                            stage_c(j - 2)
                        if j >= 3:
                            stage_d(j - 3)
                stage_b(NCH - 1)
                stage_c(NCH - 2)
                stage_c(NCH - 1)
                for j in (NCH - 3, NCH - 2, NCH - 1):
                    stage_d(j)
                flush_half(msg12b, ar_in_b, ar_out_b, "m12b")

            m1a = rpool.tile([D, CP], bf16, tag="m1a")
            nc.sync.dma_start(out=m1a[:], in_=ar_out_a[:D, :])
            m1b = rpool.tile([D, CP], bf16, tag="m1b")
            nc.sync.dma_start(out=m1b[:], in_=ar_out_b[:D, :])
            m2a = rpool.tile([D, CP], bf16, tag="m2a")
            nc.sync.dma_start(out=m2a[:], in_=ar_out_a[D:, :])
            m2b = rpool.tile([D, CP], bf16, tag="m2b")
            nc.sync.dma_start(out=m2b[:], in_=ar_out_b[D:, :])

            # =========== region 3: AR-dependent company updates =========
            with tc.tile_pool(name="ps3", bufs=1, space="PSUM") as pR3:
                msgr1 = rpool.tile([D, CP], f32, tag="msgr1")
                nc.vector.tensor_add(out=msgr1[:], in0=m1a[:], in1=m1b[:])
                nc.vector.tensor_add(out=msgr1[:], in0=msgr1[:], in1=pcorr[:])
                msgr2 = rpool.tile([D, CP], f32, tag="msgr2")
                nc.vector.tensor_add(out=msgr2[:], in0=m2a[:], in1=m2b[:])

                # layer 1 (ha precomputed in region 1.5)
                hb = gin_update(comp0T, msgr1[:], 0, 1, "b1", pR3)
                comp1T = rpool.tile([D, CP], f32, tag="comp1T")
                nc.vector.tensor_add(out=comp1T[:], in0=ha[:], in1=hb[:])
                comp1rb = t_to_rows(comp1T, "comp1r", pR3)
                # layer 2
                pc2c1 = c2c_msg(comp1rb, pR3)
                ha2 = gin_update(comp1T, pc2c1[:], 1, 0, "a2", pR3)
                hb2 = gin_update(comp1T, msgr2[:], 1, 1, "b2", pR3)
                comp2T = rpool.tile([D, CP], f32, tag="comp2T")
                nc.vector.tensor_add(out=comp2T[:], in0=ha2[:], in1=hb2[:])

                # classifier (f32)
                pcls = pR3.tile([2, CP], f32, tag="big", bufs=2)
                for sl, sz in ((0, 512), (512, 128)):
                    nc.tensor.matmul(out=pcls[:, sl:sl + sz], lhsT=clsw_s[:],
                                     rhs=comp2T[:, sl:sl + sz],
                                     start=True, stop=False)
                    nc.tensor.matmul(out=pcls[:, sl:sl + sz], lhsT=clsb_s[:],
                                     rhs=ones_r[:, sl:sl + sz],
                                     start=False, stop=True)
                outs = wk.tile([2, CP], f32, tag="outs", bufs=1)
                nc.vector.tensor_copy(out=outs[:], in_=pcls[:])
                nc.sync.dma_start(out=out_d[:], in_=outs[:])

    nc.compile()
    return nc


_CACHE = {}


def _get_program(a_vals):
    key = np.asarray(a_vals, np.float64).tobytes()
    if key not in _CACHE:
        _CACHE[key] = _build(a_vals)
    return _CACHE[key]


def _prep_inputs(inp):
    """Host-side sharding, layout swizzles, count-matrix construction."""
    sx = np.asarray(inp["sentence_x"], np.float32)
    cts = np.asarray(inp["company_ts"], np.float32)
    cids = np.asarray(inp["company_ids"]).astype(np.int64)
    emb = np.asarray(inp["comp_emb"], np.float32)

    tsq = np.zeros((CP, T), np.float32)
    tsq[:NC] = cts[:, :, F - 2]
    cembT = np.zeros((D, CP), np.float32)
    cembT[:, :NC] = emb[cids].T

    s2c_s = np.asarray(inp["ei_s2c_src"]).astype(np.int64)
    s2c_d = np.asarray(inp["ei_s2c_dst"]).astype(np.int64)
    c2s_s = np.asarray(inp["ei_c2s_src"]).astype(np.int64)
    c2s_d = np.asarray(inp["ei_c2s_dst"]).astype(np.int64)
    c2c_s = np.asarray(inp["ei_c2c_src"]).astype(np.int64)
    c2c_d = np.asarray(inp["ei_c2c_dst"]).astype(np.int64)

    c3t = np.bincount(c2c_s * CP + c2c_d, minlength=CP * CP).reshape(
        CP, CP).astype(np.float32)
    # [640, 640] -> [128, 5, 640] (partition = src % 128, block = src // 128)
    c3t8 = np.ascontiguousarray(
        c3t.reshape(5, 128, CP).transpose(1, 0, 2)).astype(F8)

    deg1 = np.bincount(s2c_d, minlength=CP).astype(np.float32).reshape(1, CP)

    core1 = s2c_s // SS
    loc1 = s2c_s - core1 * SS
    core2 = c2s_d // SS
    loc2 = c2s_d - core2 * SS

    per_core = []
    for k in range(NCORES):
        m1 = core1 == k
        cnt1 = np.bincount(loc1[m1] * CP + s2c_d[m1],
                           minlength=SP * CP).reshape(SP, CP)
        c1t = np.ascontiguousarray(
            cnt1.reshape(NSC, 4, 128, CP).transpose(2, 0, 1, 3)).astype(F8)
        del cnt1
        m2 = core2 == k
        cnt2 = np.bincount(c2s_s[m2] * SP + loc2[m2],
                           minlength=CP * SP).reshape(CP, SP)
        c2t = np.ascontiguousarray(
            cnt2.reshape(5, 128, NSC, 4, 128).transpose(1, 2, 3, 0, 4)
        ).astype(F8)
        del cnt2
        xk = np.zeros((SP, A), np.float32)
        xk[:SS] = sx[SS * k:SS * (k + 1)]
        xq = np.ascontiguousarray(
            xk.reshape(NSC, 4, 128, 6, 128).transpose(4, 0, 1, 3, 2)
        ).astype(BF)
        del xk
        per_core.append({
            "xq": xq, "c1t": c1t, "c2t": c2t,
            "tsmy": np.ascontiguousarray(tsq[CW * k:CW * (k + 1)]),
            "cembmy": np.ascontiguousarray(cembT[:, CW * k:CW * (k + 1)]),
        })

    gin_W = np.asarray(inp["gin_W"], np.float32)
    gin_b = np.asarray(inp["gin_b"], np.float32)
    proj_W = np.asarray(inp["proj_W"], np.float32)
    proj_b = np.asarray(inp["proj_b"], np.float32)
    # b02' = b02 + W02 @ proj_b (proj_b folded out of the sentence stream)
    b02p = gin_b[0, 2] + gin_W[0, 2] @ proj_b
    gw02aug = np.concatenate([gin_W[0, 2].T, b02p.reshape(1, D)],
                             axis=0).astype(BF)
    wprojr = np.ascontiguousarray(
        proj_W.T.reshape(6, 128, D).transpose(1, 0, 2)).astype(BF)

    b0 = (np.asarray(inp["lstm_bih0"], np.float32)
          + np.asarray(inp["lstm_bhh0"], np.float32))
    b1 = (np.asarray(inp["lstm_bih1"], np.float32)
          + np.asarray(inp["lstm_bhh1"], np.float32))

    lw = {
        "c3t8": c3t8, "tsq": tsq,
        "bn_g": np.asarray(inp["bn_gamma"], np.float32).reshape(T, 1),
        "bn_b": np.asarray(inp["bn_beta"], np.float32).reshape(T, 1),
        "wih0r": np.asarray(inp["lstm_Wih0"], np.float32)[:, 0].reshape(
            1, 4 * D).astype(BF),
        "b0col": np.ascontiguousarray(b0.reshape(4, D).T),
        "whh0": np.ascontiguousarray(
            np.asarray(inp["lstm_Whh0"], np.float32).T).astype(BF),
        "wih1t": np.ascontiguousarray(
            np.asarray(inp["lstm_Wih1"], np.float32).T).astype(BF),
        "b1col": np.ascontiguousarray(b1.reshape(4, D).T),
        "whh1": np.ascontiguousarray(
            np.asarray(inp["lstm_Whh1"], np.float32).T).astype(BF),
        "fcw": np.ascontiguousarray(np.asarray(inp["fc_W"], np.float32).T),
        "fcb": np.asarray(inp["fc_b"], np.float32).reshape(1, D),
        "wprojr": wprojr,
        "gw02aug": gw02aug,
        "clsw": np.ascontiguousarray(np.asarray(inp["cls_W"], np.float32).T),
        "clsb": np.asarray(inp["cls_b"], np.float32).reshape(1, 2),
        "deg1row": deg1,
        "projbcol": proj_b.reshape(1, D),
        "ones128": np.ones((128, 1), np.float32),
        "epsd": np.full((T, 1), 1e-5, np.float32),
        "onesrow": np.ones((1, CP), np.float32),
        "onesrowb": np.ones((1, CP), BF),
    }
    for l in range(2):
        for r in range(2):
            lw[f"gw{l}{r}"] = np.ascontiguousarray(gin_W[l, r].T).astype(BF)
            lw[f"gb{l}{r}"] = gin_b[l, r].reshape(1, D).astype(BF)

    in_maps = [{**per_core[k], **lw} for k in range(NCORES)]
    return in_maps


def kernel(**inputs):
    inp = {k: np.asarray(v) for k, v in inputs.items()}
    a_vals = np.asarray(inp["gin_a"], np.float32)
    nc = _get_program(a_vals)
    in_maps = _prep_inputs(inp)
    res = run_bass_kernel_spmd(nc, in_maps, list(range(NCORES)))
    out = np.asarray(res.results[0]["out"])  # [2, CP]
    return np.ascontiguousarray(out.T[:NC]).astype(np.float32)


if __name__ == "__main__":
    # quick self-test against the reference
    sys.path.insert(0, "/root/problem")
    import reference

    inputs = {k: np.asarray(v) for k, v in reference.setup_inputs().items()}
    expected = np.asarray(reference.reference(**reference.setup_inputs()))
    got = kernel(**inputs)
    err = np.abs(got - expected).max() / (np.abs(expected).max() + 1e-30)
    print("Relative error:", err)
